# revision 6
# baseline (speedup 1.0000x reference)
"""Trainium2 Bass kernel for a 6-layer transformer encoder (nn_Encoder).

Sharding across 8 NeuronCores:
  - Attention is tensor-parallel over heads: core r owns heads 2r, 2r+1 and
    computes Q/K/V + scores + softmax + att@V for those heads over ALL tokens.
  - Everything row-wise (out-projection, residuals, LayerNorms, FFN) is
    token-parallel: core r owns tokens [256r, 256r+256) of the flattened
    (batch*seq) axis, with the full Wo/W1/W2 replicated.
  - Per layer only two cheap collectives are needed:
      AllGather of the layer input x^T (bf16, split in two halves so the
      second half overlaps the first half's QKV matmuls)
      AllToAll of per-head attention outputs   -> head-sharded -> token-sharded
    (No AllReduce anywhere: contractions over sharded axes are avoided by
    construction.)

Activations live transposed ("T-layout", [feature, token]) so every matmul
consumes natural-layout weights as the stationary operand. LayerNorm
reductions over D (the partition axis) use ones-matmuls on the PE;
mean/rstd are broadcast back across partitions with gpsimd.partition_broadcast.
Softmax skips max-subtraction (scores are bounded, |S|<6) and its denominator
is computed for free by appending a ones-column to V in the att@V matmul.

dtypes: bf16 for the big GEMMs (QKV / out-proj / FFN weights and moving
operands; fp32 PSUM accumulate), float32r (full-speed fp32, ~1e-4) for
scores / att@V / LN statistics.
"""
import numpy as np
import ml_dtypes

L, H, D, DK, F = 6, 16, 1024, 64, 4096
B, S = 2, 1024
NC = 8
TOK = (B * S) // NC  # 256 tokens per core
HPC = H // NC        # 2 heads per core
EPS = 1e-5
NPBF16 = ml_dtypes.bfloat16

_CACHE = {}


def _build_program():
    import concourse.bacc as bacc
    import concourse.tile as tile
    import concourse.mybir as mybir
    from contextlib import ExitStack

    FP32 = mybir.dt.float32
    FP32R = mybir.dt.float32r
    BF = mybir.dt.bfloat16
    AF = mybir.ActivationFunctionType
    ALU = mybir.AluOpType

    nc = bacc.Bacc(
        "TRN2",
        target_bir_lowering=False,
        debug=False,
        enable_asserts=False,
        num_devices=NC,
    )

    # ---------------- external I/O ----------------
    x0all_e = nc.dram_tensor("x0all", [D, B * S], BF, kind="ExternalInput")
    x0mine_e = nc.dram_tensor("x0mine", [D, TOK], FP32, kind="ExternalInput")
    wq_e = nc.dram_tensor("wq", [L, 128, 1024], BF, kind="ExternalInput")
    wk_e = nc.dram_tensor("wk", [L, 128, 1024], BF, kind="ExternalInput")
    wv_e = nc.dram_tensor("wv", [L, 128, 1024], BF, kind="ExternalInput")
    bq_e = nc.dram_tensor("bq", [L, 128, 1], FP32, kind="ExternalInput")
    bk_e = nc.dram_tensor("bk", [L, 128, 1], FP32, kind="ExternalInput")
    bv_e = nc.dram_tensor("bv", [L, 128, 1], FP32, kind="ExternalInput")
    wo_e = nc.dram_tensor("wo", [L, 8, 128, 1024], BF, kind="ExternalInput")
    bo_e = nc.dram_tensor("bo", [L, 128, 8], FP32, kind="ExternalInput")
    w1_e = nc.dram_tensor("w1", [L, 32, 128, 1024], BF, kind="ExternalInput")
    b1_e = nc.dram_tensor("b1", [L, 128, 32], FP32, kind="ExternalInput")
    w2_e = nc.dram_tensor("w2", [L, 8, 2, 128, 2048], BF, kind="ExternalInput")
    b2_e = nc.dram_tensor("b2", [L, 128, 8], FP32, kind="ExternalInput")
    g1_e = nc.dram_tensor("g1", [L, 128, 8], FP32, kind="ExternalInput")
    be1_e = nc.dram_tensor("be1", [L, 128, 8], FP32, kind="ExternalInput")
    g2_e = nc.dram_tensor("g2", [L, 128, 8], FP32, kind="ExternalInput")
    be2_e = nc.dram_tensor("be2", [L, 128, 8], FP32, kind="ExternalInput")
    ident_e = nc.dram_tensor("ident", [128, 128], FP32, kind="ExternalInput")
    out_e = nc.dram_tensor("out_xT", [D, TOK], FP32, kind="ExternalOutput")

    RG = [list(range(NC))]

    with tile.TileContext(nc) as tc, ExitStack() as ctx:
        P = lambda name, bufs, **kw: ctx.enter_context(
            tc.tile_pool(name=name, bufs=bufs, **kw)
        )
        p_xg = P("xg", 8)        # [128,1024] bf16: x^T d-tiles (one token-half)
        p_qk = P("qk", 2)        # qt/kt [128,2048] fp32r
        p_vtT = P("vtT", 2)      # [128,512] fp32r v^T staging
        p_vaug = P("vaug", 1)    # [128,2080] fp32r: v-natural + ones cols
        p_exp = P("exp", 4)      # [128,1024] fp32r exp(scores^T), 2 heads packed
        p_ota = P("ota", 1)      # [128,2048] bf16: o^T (my heads, all tokens)
        p_otf = P("otf", 1)      # [128,2048] bf16: o^T (all heads, my tokens)
        p_xst = P("xst", 2)      # x_mine [128,2048] fp32r (rotates per layer)
        p_z = P("z", 1)          # residual sums [128,2048] fp32r
        p_xp = P("xp", 1)        # x' [128,2048] fp32r
        p_xpb = P("xpb", 1)      # x' [128,2048] bf16
        p_x2b = P("x2b", 1)      # x'' [128,2048] bf16 (AG payload)
        p_ht = P("ht", 1)        # [128,8192] bf16: FFN hidden (32 x 256)
        p_wqkv = P("wqkv", 3)    # [128,1024] bf16 (3/layer, tag-shared)
        p_wo = P("wo", 8)        # [128,1024] bf16 (8/layer resident)
        p_w1 = P("w1", 3)        # [128,2048] bf16 stream (2 f-chunks per tile)
        p_w2 = P("w2", 4)        # [128,2048] bf16 stream
        p_bias = P("bias", 2)    # small per-layer bias tiles
        p_stat = P("stat", 4)    # [1,N] stats
        p_bc = P("bc", 4)        # broadcast tiles
        p_sq = P("sq", 3)        # z^2 staging [128,256]
        p_tmp = P("tmp", 4)      # [128,256] fp32r temps
        ps_big = P("ps_big", 2, space="PSUM")  # [128,1024] 2-bank slots (shared tag)
        ps_b = P("ps_b", 4, space="PSUM")    # attV [65,512] / transpose / LN stats
        d_agi = P("d_agi", 2, space="DRAM")
        d_ago = P("d_ago", 8, space="DRAM")
        d_a2i = P("d_a2i", 2, space="DRAM")
        d_a2o = P("d_a2o", 2, space="DRAM")

        # constants
        p_const = ctx.enter_context(tc.tile_pool(name="const", bufs=1))
        ones_f = p_const.tile([128, 1], FP32, name="ones_f", tag="onesf")
        nc.gpsimd.memset(ones_f[:], 1.0)
        ones_sb = p_const.tile([128, 1], FP32R, name="ones_sb", tag="ones")
        nc.scalar.activation(ones_sb[:], ones_f[:], AF.Copy, bias=0.0, scale=1.0)
        eps_sb = p_const.tile([1, 1], FP32, name="eps_sb", tag="eps")
        nc.gpsimd.memset(eps_sb[:], float(EPS))
        ident_sb = p_bias.tile([128, 128], FP32R, name="ident_sb", bufs=1)
        nc.sync.dma_start(ident_sb[:], ident_e[:].bitcast(FP32R))

        # initial x_mine (fp32 residual basis)
        x_mine = p_xst.tile([128, 8 * TOK], FP32R, name="x_mine", tag="xst")
        for dc in range(8):
            nc.sync.dma_start(
                x_mine[:, TOK * dc : TOK * (dc + 1)],
                x0mine_e[128 * dc : 128 * (dc + 1), :].bitcast(FP32R),
            )

        ag_halves = None  # two DRAM tensors [NC, 512, TOK] from the split AG

        ln_env = dict(
            nc=nc, mybir=mybir, ones=ones_sb, eps=eps_sb,
            p_sq=p_sq, p_stat=p_stat, p_bc=p_bc, p_tmp=p_tmp, ps_st=ps_b,
        )

        for l in range(L):
            # -- per-layer weight/bias loads (emitted early: prefetch) --
            wq_sb = p_wqkv.tile([128, 1024], BF, name="wq_sb", tag="wqkv")
            nc.sync.dma_start(wq_sb[:], wq_e[l])
            wk_sb = p_wqkv.tile([128, 1024], BF, name="wk_sb", tag="wqkv")
            nc.sync.dma_start(wk_sb[:], wk_e[l])
            wv_sb = p_wqkv.tile([128, 1024], BF, name="wv_sb", tag="wqkv")
            nc.sync.dma_start(wv_sb[:], wv_e[l])
            bq_sb = p_bias.tile([128, 1], FP32, name="bq_sb", tag="bq")
            nc.sync.dma_start(bq_sb[:], bq_e[l])
            bk_sb = p_bias.tile([128, 1], FP32, name="bk_sb", tag="bk")
            nc.sync.dma_start(bk_sb[:], bk_e[l])
            bv_sb = p_bias.tile([128, 1], FP32, name="bv_sb", tag="bv")
            nc.sync.dma_start(bv_sb[:], bv_e[l])
            wo_sb = []
            for t in range(8):
                w = p_wo.tile([128, 1024], BF, name=f"wo{t}", tag="wo")
                nc.sync.dma_start(w[:], wo_e[l, t])
                wo_sb.append(w)
            bo_sb = p_bias.tile([128, 8], FP32, name="bo_sb", tag="bo")
            nc.sync.dma_start(bo_sb[:], bo_e[l])
            g1_sb = p_bias.tile([128, 8], FP32, name="g1_sb", tag="g1")
            nc.sync.dma_start(g1_sb[:], g1_e[l])
            be1_sb = p_bias.tile([128, 8], FP32, name="be1_sb", tag="be1")
            nc.sync.dma_start(be1_sb[:], be1_e[l])
            b1_sb = p_bias.tile([128, 32], FP32, name="b1_sb", tag="b1")
            nc.sync.dma_start(b1_sb[:], b1_e[l])
            b2_sb = p_bias.tile([128, 8], FP32, name="b2_sb", tag="b2")
            nc.sync.dma_start(b2_sb[:], b2_e[l])
            g2_sb = p_bias.tile([128, 8], FP32, name="g2_sb", tag="g2")
            nc.sync.dma_start(g2_sb[:], g2_e[l])
            be2_sb = p_bias.tile([128, 8], FP32, name="be2_sb", tag="be2")
            nc.sync.dma_start(be2_sb[:], be2_e[l])

            # ---------- QKV ----------
            # bf16 for q/k/exp/vaug: HW runs fp32r matmuls in fp32_mode=HIGH
            # (~4 cyc/row); bf16 is 1 cyc/row and the 2e-2 tolerance allows it.
            qt = p_qk.tile([128, 2048], BF, name="qt", tag="qk")
            kt = p_qk.tile([128, 2048], BF, name="kt", tag="qk")
            vaug = p_vaug.tile([128, 16 * (2 * DK + 2)], BF, name="vaug")
            VW = 2 * DK + 2  # 130: per t-tile block [h0(64)|ones|h1(64)|ones]

            for half in range(2):
                # xg: one token-half (= one batch) of full x^T, 8 d-tiles
                xg = []
                for dt in range(8):
                    t = p_xg.tile([128, 1024], BF, name=f"xg{dt}", tag="xg")
                    if l == 0:
                        nc.sync.dma_start(
                            t[:],
                            x0all_e[
                                128 * dt : 128 * (dt + 1),
                                1024 * half : 1024 * (half + 1),
                            ],
                        )
                    else:
                        # token-half `half` = rank blocks 4*half..4*half+3;
                        # d-quarter dt//2 comes from sub-AllGather dt//2
                        nc.sync.dma_start(
                            t[:].rearrange("p (r s) -> p r s", r=4),
                            ag_halves[dt // 2][
                                4 * half : 4 * half + 4,
                                128 * (dt % 2) : 128 * (dt % 2 + 1),
                                :,
                            ].rearrange("r p s -> p r s"),
                        )
                    xg.append(t)
                for sc2 in range(2):
                    sc = 2 * half + sc2
                    ssl = slice(512 * sc, 512 * (sc + 1))  # global token chunk
                    hsl = slice(512 * sc2, 512 * (sc2 + 1))  # within-half chunk
                    q_ps = ps_big.tile([128, 512], FP32, name="q_ps", tag="big")
                    for dt in range(8):
                        nc.tensor.matmul(
                            q_ps[:],
                            wq_sb[:, 128 * dt : 128 * (dt + 1)],
                            xg[dt][:, hsl],
                            start=(dt == 0),
                            stop=(dt == 7),
                        )
                    nc.vector.tensor_scalar_add(qt[:, ssl], q_ps[:], bq_sb[:])
                    k_ps = ps_big.tile([128, 512], FP32, name="k_ps", tag="big")
                    for dt in range(8):
                        nc.tensor.matmul(
                            k_ps[:],
                            wk_sb[:, 128 * dt : 128 * (dt + 1)],
                            xg[dt][:, hsl],
                            start=(dt == 0),
                            stop=(dt == 7),
                        )
                    nc.vector.tensor_scalar_add(kt[:, ssl], k_ps[:], bk_sb[:])
                    v_ps = ps_big.tile([128, 512], FP32, name="v_ps", tag="big")
                    for dt in range(8):
                        nc.tensor.matmul(
                            v_ps[:],
                            wv_sb[:, 128 * dt : 128 * (dt + 1)],
                            xg[dt][:, hsl],
                            start=(dt == 0),
                            stop=(dt == 7),
                        )
                    vtT = p_vtT.tile([128, 512], FP32R, name="vtT", tag="vtT")
                    nc.vector.tensor_scalar_add(vtT[:], v_ps[:], bv_sb[:])
                    for j in range(4):
                        tt = 4 * sc + j  # global t-tile 0..15
                        tr_ps = ps_b.tile([128, 128], FP32, name="tr_ps", tag="ps")
                        nc.tensor.transpose(
                            tr_ps[:].bitcast(FP32R),
                            vtT[:, 128 * j : 128 * (j + 1)],
                            ident_sb[:],
                        )
                        o = VW * tt
                        nc.vector.tensor_copy(vaug[:, o : o + 64], tr_ps[:, 0:64])
                        nc.vector.tensor_copy(
                            vaug[:, o + 65 : o + 129], tr_ps[:, 64:128]
                        )
                        nc.vector.tensor_copy(vaug[:, o + 64 : o + 65], ones_sb[:])
                        nc.vector.tensor_copy(
                            vaug[:, o + 129 : o + 130], ones_sb[:]
                        )

            # ---------- attention ----------
            # Two independent 512-token streams (the sc chunks of one batch)
            # interleave so the PE never waits on the scores->exp->attV chain.
            ota = p_ota.tile([128, 2048], BF, name="ota")
            a2i = d_a2i.tile([NC, 128, TOK], BF, name="a2i", tag="a2i")
            for b in range(B):
                base = S * b
                ssls = [
                    slice(base + 512 * sc, base + 512 * (sc + 1)) for sc in range(2)
                ]
                o_ps = {
                    (sc, h): ps_b.tile([65, 512], FP32, name=f"o_ps{sc}{h}", tag="ps")
                    for sc in range(2)
                    for h in range(HPC)
                }
                e_prev = {0: None, 1: None}
                for jt in range(9):
                    for sc in range(2):
                        if jt < 8:
                            tsl = slice(base + 128 * jt, base + 128 * (jt + 1))
                            # both heads' scores into one 2-bank PSUM tile,
                            # then a single [128,1024] exp on ACT
                            s2 = ps_big.tile(
                                [128, 1024], FP32, name="s2", tag="big"
                            )
                            for h in range(HPC):
                                hp = slice(64 * h, 64 * (h + 1))
                                nc.tensor.matmul(
                                    s2[:, 512 * h : 512 * (h + 1)],
                                    kt[hp, tsl],
                                    qt[hp, ssls[sc]],
                                    start=True,
                                    stop=True,
                                )
                            e2 = p_exp.tile(
                                [128, 1024], BF, name="e2", tag="e"
                            )
                            nc.scalar.activation(
                                e2[:], s2[:], AF.Exp, bias=0.0, scale=0.125
                            )
                        if jt > 0:
                            pj = jt - 1
                            for h in range(HPC):
                                o = VW * (8 * b + pj) + (DK + 1) * h
                                nc.tensor.matmul(
                                    o_ps[(sc, h)][:],
                                    vaug[:, o : o + 65],
                                    e_prev[sc][:, 512 * h : 512 * (h + 1)],
                                    start=(pj == 0),
                                    stop=(pj == 7),
                                )
                        e_prev[sc] = e2 if jt < 8 else None
                for sc in range(2):
                    for h in range(HPC):
                        drow = p_stat.tile(
                            [1, 512], FP32, name="drow", tag="drow", bufs=2
                        )
                        nc.vector.tensor_copy(drow[:], o_ps[(sc, h)][64:65, :])
                        den = p_stat.tile(
                            [1, 512], FP32, name="den", tag="den", bufs=2
                        )
                        nc.vector.reciprocal_approx_fast(den[:], drow[:])
                        den_bc = p_bc.tile(
                            [64, 512], FP32, name="den_bc", tag="dbc", bufs=2
                        )
                        nc.gpsimd.partition_broadcast(den_bc[:], den[:])
                        nc.vector.tensor_mul(
                            ota[64 * h : 64 * (h + 1), ssls[sc]],
                            o_ps[(sc, h)][0:64, :],
                            den_bc[:].bitcast(FP32R),
                        )
                    # ship this 512-token quadrant (= 2 rank chunks) to the
                    # A2A bounce as soon as both heads are normalized
                    rr = 2 * (2 * b + sc)
                    nc.sync.dma_start(
                        a2i[rr : rr + 2].rearrange("r p s -> p r s"),
                        ota[:, ssls[sc]].rearrange("p (r s) -> p r s", r=2),
                    )

            # ---------- AllToAll + out-projection + LN1 ----------
            a2o = d_a2o.tile([NC, 128, TOK], BF, name="a2o", tag="a2o")
            nc.gpsimd.collective_compute(
                "AllToAll",
                ALU.bypass,
                replica_groups=RG,
                ins=[a2i[:].opt()],
                outs=[a2o[:].opt()],
            )
            otf = p_otf.tile([128, 8 * TOK], BF, name="otf")
            for t in range(NC):
                nc.sync.dma_start(otf[:, TOK * t : TOK * (t + 1)], a2o[t])

            z1 = p_z.tile([128, 8 * TOK], FP32R, name="z1", tag="z")
            sum1_ps = ps_b.tile([1, TOK], FP32, name="sum1_ps", tag="ps")
            sq1_ps = ps_b.tile([1, TOK], FP32, name="sq1_ps", tag="ps")
            for dc in range(8):
                dsl = slice(TOK * dc, TOK * (dc + 1))
                y_ps = ps_big.tile([128, TOK], FP32, name="y_ps", tag="big")
                for t in range(8):
                    nc.tensor.matmul(
                        y_ps[:],
                        wo_sb[t][:, 128 * dc : 128 * (dc + 1)],
                        otf[:, TOK * t : TOK * (t + 1)],
                        start=(t == 0),
                        stop=(t == 7),
                    )
                nc.vector.scalar_tensor_tensor(
                    z1[:, dsl], y_ps[:], bo_sb[:, dc : dc + 1], x_mine[:, dsl],
                    ALU.add, ALU.add,
                )
                # interleaved LN1 statistics
                nc.tensor.matmul(
                    sum1_ps[:], ones_sb[:], z1[:, dsl],
                    start=(dc == 0), stop=(dc == 7),
                )
                zsq = p_sq.tile([128, TOK], FP32R, name="zsq", tag="sq")
                nc.vector.tensor_mul(zsq[:], z1[:, dsl], z1[:, dsl])
                nc.tensor.matmul(
                    sq1_ps[:], ones_sb[:], zsq[:],
                    start=(dc == 0), stop=(dc == 7),
                )

            xp = p_xp.tile([128, 8 * TOK], FP32R, name="xp")
            xpb = p_xpb.tile([128, 8 * TOK], BF, name="xpb")
            _emit_layernorm(ln_env, z1, g1_sb, be1_sb, xp, xpb, sum1_ps, sq1_ps)

            # ---------- FFN + LN2 (+ split AllGather for next layer) ----------
            ht = p_ht.tile([128, 32 * TOK], BF, name="ht")
            for g in range(16):
                w1t = p_w1.tile([128, 2048], BF, name="w1t", tag="w1")
                nc.sync.dma_start(
                    w1t[:].rearrange("p (c j) -> p c j", c=2),
                    w1_e[l, 2 * g : 2 * g + 2].rearrange("c p j -> p c j"),
                )
                for c in range(2):
                    fc = 2 * g + c
                    h_ps = ps_big.tile([128, TOK], FP32, name="h_ps", tag="big")
                    for dt in range(8):
                        nc.tensor.matmul(
                            h_ps[:],
                            w1t[:, 1024 * c + 128 * dt : 1024 * c + 128 * (dt + 1)],
                            xpb[:, TOK * dt : TOK * (dt + 1)],
                            start=(dt == 0),
                            stop=(dt == 7),
                        )
                    # relu(h + b1) on DVE (keeps ACT free for attention exp)
                    nc.vector.tensor_scalar(
                        ht[:, TOK * fc : TOK * (fc + 1)],
                        h_ps[:],
                        b1_sb[:, fc : fc + 1],
                        0.0,
                        ALU.add,
                        ALU.max,
                    )

            z2 = p_z.tile([128, 8 * TOK], FP32R, name="z2", tag="z")
            sum2_ps = ps_b.tile([1, TOK], FP32, name="sum2_ps", tag="ps")
            sq2_ps = ps_b.tile([1, TOK], FP32, name="sq2_ps", tag="ps")
            for dc in range(8):
                dsl = slice(TOK * dc, TOK * (dc + 1))
                y2_ps = ps_big.tile([128, TOK], FP32, name="y2_ps", tag="big")
                for half in range(2):
                    w2t = p_w2.tile([128, 2048], BF, name="w2t", tag="w2")
                    nc.sync.dma_start(w2t[:], w2_e[l, dc, half])
                    for ft in range(16):
                        gt = 16 * half + ft
                        nc.tensor.matmul(
                            y2_ps[:],
                            w2t[:, 128 * ft : 128 * (ft + 1)],
                            ht[:, TOK * gt : TOK * (gt + 1)],
                            start=(gt == 0),
                            stop=(gt == 31),
                        )
                nc.vector.scalar_tensor_tensor(
                    z2[:, dsl], y2_ps[:], b2_sb[:, dc : dc + 1], xp[:, dsl],
                    ALU.add, ALU.add,
                )
                # interleaved LN2 statistics
                nc.tensor.matmul(
                    sum2_ps[:], ones_sb[:], z2[:, dsl],
                    start=(dc == 0), stop=(dc == 7),
                )
                zsq2 = p_sq.tile([128, TOK], FP32R, name="zsq2", tag="sq")
                nc.vector.tensor_mul(zsq2[:], z2[:, dsl], z2[:, dsl])
                nc.tensor.matmul(
                    sq2_ps[:], ones_sb[:], zsq2[:],
                    start=(dc == 0), stop=(dc == 7),
                )

            x_mine = p_xst.tile([128, 8 * TOK], FP32R, name="x_mine", tag="xst")
            if l < L - 1:
                x2b = p_x2b.tile([128, 8 * TOK], BF, name="x2b")
                agi = d_agi.tile([D, TOK], BF, name="agi", tag="agi")
                agos = [
                    d_ago.tile(
                        [NC, 256, TOK], BF, name=f"ago{q}", tag=f"ago{q}",
                        addr_space="Shared",
                    )
                    for q in range(4)
                ]

                def post_dc(dc, _x2b=x2b, _agi=agi, _agos=agos):
                    # ship each normalized d-chunk to the AG bounce as it
                    # completes; fire a quarter-AllGather after each odd dc so
                    # transfer pipelines with normalize and next-layer QKV
                    nc.sync.dma_start(
                        _agi[128 * dc : 128 * (dc + 1), :],
                        _x2b[:, TOK * dc : TOK * (dc + 1)],
                    )
                    if dc % 2 == 1:
                        q = dc // 2
                        nc.gpsimd.collective_compute(
                            "AllGather",
                            ALU.bypass,
                            replica_groups=RG,
                            ins=[_agi[256 * q : 256 * (q + 1), :].opt()],
                            outs=[_agos[q][:].opt()],
                        )

                _emit_layernorm(
                    ln_env, z2, g2_sb, be2_sb, x_mine, x2b, sum2_ps, sq2_ps,
                    post_dc=post_dc,
                )
                ag_halves = agos
            else:
                _emit_layernorm(ln_env, z2, g2_sb, be2_sb, x_mine, None, sum2_ps, sq2_ps)
                for dc in range(8):
                    nc.sync.dma_start(
                        out_e[128 * dc : 128 * (dc + 1), :].bitcast(FP32R),
                        x_mine[:, TOK * dc : TOK * (dc + 1)],
                    )

    nc.compile()
    return nc


def _emit_layernorm(env, z, g_sb, b_sb, out_fp, out_bf, sum_ps, sq_ps,
                    post_dc=None):
    """LayerNorm over D; sum/sumsq PSUM stats are pre-accumulated by caller."""
    nc = env["nc"]
    mybir = env["mybir"]
    FP32 = mybir.dt.float32
    FP32R = mybir.dt.float32r
    AF = mybir.ActivationFunctionType
    ALU = mybir.AluOpType
    eps_sb = env["eps"]
    p_stat, p_bc, p_tmp = env["p_stat"], env["p_bc"], env["p_tmp"]

    st = lambda nm: p_stat.tile([1, TOK], FP32, name=nm, tag="lnstat", bufs=8)
    mu = st("mu")
    nc.vector.tensor_scalar_mul(mu[:], sum_ps[:], 1.0 / D)
    ex2 = st("ex2")
    nc.vector.tensor_scalar_mul(ex2[:], sq_ps[:], 1.0 / D)
    var = st("var")
    musq = st("musq")
    nc.vector.tensor_mul(musq[:], mu[:], mu[:])
    nc.vector.tensor_sub(var[:], ex2[:], musq[:])
    std = st("std")
    nc.scalar.activation(std[:], var[:], AF.Sqrt, bias=eps_sb[:], scale=1.0)
    rsig = st("rsig")
    nc.vector.reciprocal_approx_fast(rsig[:], std[:])
    mu_bc = p_bc.tile([128, TOK], FP32, name="mu_bc", tag="mu_bc", bufs=2)
    nc.gpsimd.partition_broadcast(mu_bc[:], mu[:])
    rs_bc = p_bc.tile([128, TOK], FP32, name="rs_bc", tag="rs_bc", bufs=2)
    nc.gpsimd.partition_broadcast(rs_bc[:], rsig[:])
    for dc in range(8):
        dsl = slice(TOK * dc, TOK * (dc + 1))
        t1 = p_tmp.tile([128, TOK], FP32R, name="t1", tag="tmp")
        nc.vector.tensor_sub(t1[:], z[:, dsl], mu_bc[:].bitcast(FP32R))
        t2 = p_tmp.tile([128, TOK], FP32R, name="t2", tag="tmp")
        nc.vector.tensor_mul(t2[:], t1[:], rs_bc[:].bitcast(FP32R))
        nc.vector.tensor_scalar(
            out_fp[:, dsl],
            t2[:],
            g_sb[:, dc : dc + 1],
            b_sb[:, dc : dc + 1],
            ALU.mult,
            ALU.add,
        )
        if out_bf is not None:
            nc.vector.tensor_copy(out_bf[:, dsl], out_fp[:, dsl])
        if post_dc is not None:
            post_dc(dc)


def _pack_inputs(src, Wq, bq, Wk, bk, Wv, bv, Wo, bo, ln1_g, ln1_b,
                 W1, b1, W2, b2, ln2_g, ln2_b):
    """Host-side sharding/packing. Returns list of per-core input dicts."""
    f32 = np.float32
    # positional encoding (phase == pos, since floor(dim/D) == 0)
    pos = np.arange(S, dtype=f32).reshape(-1, 1)
    dim = np.arange(D)
    pe = np.where(dim[None, :] % 2 == 0, np.sin(pos), np.cos(pos)).astype(f32)
    x0 = (np.asarray(src, f32) + pe[None]).reshape(B * S, D)
    x0T = np.ascontiguousarray(x0.T)                      # [D, 2048]
    x0T_bf = x0T.astype(NPBF16)

    bf = lambda a: np.ascontiguousarray(a).astype(NPBF16)
    fc = lambda a: np.ascontiguousarray(a).astype(f32)

    Wo_p = bf(np.asarray(Wo, f32).reshape(L, 8, 128, 1024))
    bo_p = fc(np.asarray(bo, f32).reshape(L, 8, 128).transpose(0, 2, 1))
    g1_p = fc(np.asarray(ln1_g, f32).reshape(L, 8, 128).transpose(0, 2, 1))
    be1_p = fc(np.asarray(ln1_b, f32).reshape(L, 8, 128).transpose(0, 2, 1))
    g2_p = fc(np.asarray(ln2_g, f32).reshape(L, 8, 128).transpose(0, 2, 1))
    be2_p = fc(np.asarray(ln2_b, f32).reshape(L, 8, 128).transpose(0, 2, 1))
    b1_p = fc(np.asarray(b1, f32).reshape(L, 32, 128).transpose(0, 2, 1))
    b2_p = fc(np.asarray(b2, f32).reshape(L, 8, 128).transpose(0, 2, 1))
    # W1: [L, D, F] -> [L, fc, p, (dt j)]
    W1_p = bf(
        np.asarray(W1, f32)
        .reshape(L, 8, 128, 32, 128)
        .transpose(0, 3, 2, 1, 4)
        .reshape(L, 32, 128, 1024)
    )
    # W2: [L, F, D] -> [L, dc, half, p, (ft j)]
    W2_p = bf(
        np.asarray(W2, f32)
        .reshape(L, 32, 128, 8, 128)
        .transpose(0, 3, 1, 2, 4)      # [L, dc, ft, p, j]
        .reshape(L, 8, 2, 16, 128, 128)
        .transpose(0, 1, 2, 4, 3, 5)   # [L, dc, half, p, ft, j]
        .reshape(L, 8, 2, 128, 2048)
    )
    ident = np.eye(128, dtype=f32)

    Wq = np.asarray(Wq, f32)
    Wk = np.asarray(Wk, f32)
    Wv = np.asarray(Wv, f32)
    bq = np.asarray(bq, f32)
    bk = np.asarray(bk, f32)
    bv = np.asarray(bv, f32)

    def pack_headw(Wx, r):
        # [L, D, 128] for heads 2r, 2r+1 -> [L, 128, (dt j)]
        cat = np.concatenate([Wx[:, 2 * r], Wx[:, 2 * r + 1]], axis=2)  # [L,D,128]
        return bf(
            cat.reshape(L, 8, 128, 128).transpose(0, 2, 1, 3).reshape(L, 128, 1024)
        )

    in_maps = []
    for r in range(NC):
        m = {
            "x0all": x0T_bf,
            "x0mine": np.ascontiguousarray(x0T[:, TOK * r : TOK * (r + 1)]),
            "wq": pack_headw(Wq, r),
            "wk": pack_headw(Wk, r),
            "wv": pack_headw(Wv, r),
            "bq": fc(np.concatenate([bq[:, 2 * r], bq[:, 2 * r + 1]], axis=1))[
                :, :, None
            ],
            "bk": fc(np.concatenate([bk[:, 2 * r], bk[:, 2 * r + 1]], axis=1))[
                :, :, None
            ],
            "bv": fc(np.concatenate([bv[:, 2 * r], bv[:, 2 * r + 1]], axis=1))[
                :, :, None
            ],
            "wo": Wo_p,
            "bo": bo_p,
            "w1": W1_p,
            "b1": b1_p,
            "w2": W2_p,
            "b2": b2_p,
            "g1": g1_p,
            "be1": be1_p,
            "g2": g2_p,
            "be2": be2_p,
            "ident": ident,
        }
        in_maps.append(m)
    return in_maps


def run(inputs, trace=False, trace_kwargs=None):
    """Build (cached), execute on 8 cores, return (output, BassKernelResults)."""
    from concourse.bass_utils import run_bass_kernel_spmd

    if "prog" not in _CACHE:
        _CACHE["prog"] = _build_program()
    nc = _CACHE["prog"]
    in_maps = _pack_inputs(**inputs)
    res = run_bass_kernel_spmd(
        nc, in_maps, list(range(NC)), trace=trace, **(trace_kwargs or {})
    )
    xT = np.empty((B * S, D), np.float32)
    for r in range(NC):
        xT[TOK * r : TOK * (r + 1)] = res.results[r]["out_xT"].T
    return xT.reshape(B, S, D), res


def kernel(**inputs):
    out, _ = run(inputs, trace=False)
    return out



# revision 17
# speedup vs baseline: 1.0736x; 1.0736x over previous
"""Trainium2 Bass kernel for a 6-layer transformer encoder (nn_Encoder).

Sharding across 8 NeuronCores:
  - Attention is tensor-parallel over heads: core r owns heads 2r, 2r+1 and
    computes Q/K/V + scores + softmax + att@V for those heads over ALL tokens.
  - Everything row-wise (out-projection, residuals, LayerNorms, FFN) is
    token-parallel: core r owns tokens [256r, 256r+256) of the flattened
    (batch*seq) axis, with the full Wo/W1/W2 replicated.
  - Per layer only two cheap collectives are needed:
      AllGather of the layer input x^T (bf16, split in two halves so the
      second half overlaps the first half's QKV matmuls)
      AllToAll of per-head attention outputs   -> head-sharded -> token-sharded
    (No AllReduce anywhere: contractions over sharded axes are avoided by
    construction.)

Activations live transposed ("T-layout", [feature, token]) so every matmul
consumes natural-layout weights as the stationary operand. LayerNorm
reductions over D (the partition axis) use ones-matmuls on the PE;
mean/rstd are broadcast back across partitions with gpsimd.partition_broadcast.
Softmax skips max-subtraction (scores are bounded, |S|<6) and its denominator
is computed for free by appending a ones-column to V in the att@V matmul.

dtypes: bf16 for the big GEMMs (QKV / out-proj / FFN weights and moving
operands; fp32 PSUM accumulate), float32r (full-speed fp32, ~1e-4) for
scores / att@V / LN statistics.
"""
import numpy as np
import ml_dtypes

L, H, D, DK, F = 6, 16, 1024, 64, 4096
B, S = 2, 1024
NC = 8
TOK = (B * S) // NC  # 256 tokens per core
HPC = H // NC        # 2 heads per core
EPS = 1e-5
NPBF16 = ml_dtypes.bfloat16

_CACHE = {}


def _build_program():
    import concourse.bacc as bacc
    import concourse.tile as tile
    import concourse.mybir as mybir
    from contextlib import ExitStack

    FP32 = mybir.dt.float32
    FP32R = mybir.dt.float32r
    BF = mybir.dt.bfloat16
    AF = mybir.ActivationFunctionType
    ALU = mybir.AluOpType

    nc = bacc.Bacc(
        "TRN2",
        target_bir_lowering=False,
        debug=False,
        enable_asserts=False,
        num_devices=NC,
    )

    # ---------------- external I/O ----------------
    x0all_e = nc.dram_tensor("x0all", [D, B * S], BF, kind="ExternalInput")
    x0mine_e = nc.dram_tensor("x0mine", [D, TOK], FP32, kind="ExternalInput")
    wq_e = nc.dram_tensor("wq", [L, 128, 1024], BF, kind="ExternalInput")
    wk_e = nc.dram_tensor("wk", [L, 128, 1024], BF, kind="ExternalInput")
    wv_e = nc.dram_tensor("wv", [L, 128, 1024], BF, kind="ExternalInput")
    bq_e = nc.dram_tensor("bq", [L, 128, 1], FP32, kind="ExternalInput")
    bk_e = nc.dram_tensor("bk", [L, 128, 1], FP32, kind="ExternalInput")
    bv_e = nc.dram_tensor("bv", [L, 128, 1], FP32, kind="ExternalInput")
    wo_e = nc.dram_tensor("wo", [L, 8, 128, 1024], BF, kind="ExternalInput")
    bo_e = nc.dram_tensor("bo", [L, 128, 8], FP32, kind="ExternalInput")
    w1_e = nc.dram_tensor("w1", [L, 32, 128, 1024], BF, kind="ExternalInput")
    b1_e = nc.dram_tensor("b1", [L, 128, 32], FP32, kind="ExternalInput")
    w2_e = nc.dram_tensor("w2", [L, 8, 2, 128, 2048], BF, kind="ExternalInput")
    b2_e = nc.dram_tensor("b2", [L, 128, 8], FP32, kind="ExternalInput")
    g1_e = nc.dram_tensor("g1", [L, 128, 8], FP32, kind="ExternalInput")
    be1_e = nc.dram_tensor("be1", [L, 128, 8], FP32, kind="ExternalInput")
    g2_e = nc.dram_tensor("g2", [L, 128, 8], FP32, kind="ExternalInput")
    be2_e = nc.dram_tensor("be2", [L, 128, 8], FP32, kind="ExternalInput")
    ident_e = nc.dram_tensor("ident", [128, 128], FP32, kind="ExternalInput")
    out_e = nc.dram_tensor("out_xT", [D, TOK], FP32, kind="ExternalOutput")

    RG = [list(range(NC))]

    with tile.TileContext(nc) as tc, ExitStack() as ctx:
        P = lambda name, bufs, **kw: ctx.enter_context(
            tc.tile_pool(name=name, bufs=bufs, **kw)
        )
        p_xg = P("xg", 8)        # [128,1024] bf16: x^T d-tiles (one token-half)
        p_qk = P("qk", 2)        # qt/kt [128,2048] fp32r
        p_vtT = P("vtT", 2)      # [128,512] fp32r v^T staging
        p_vaug = P("vaug", 1)    # [128,2080] fp32r: v-natural + ones cols
        p_exp = P("exp", 4)      # [128,1024] fp32r exp(scores^T), 2 heads packed
        p_ota = P("ota", 1)      # [128,2048] bf16: o^T (my heads, all tokens)
        p_otf = P("otf", 1)      # [128,2048] bf16: o^T (all heads, my tokens)
        p_xst = P("xst", 2)      # x_mine [128,2048] fp32r (rotates per layer)
        p_z = P("z", 1)          # residual sums [128,2048] fp32r
        p_xp = P("xp", 1)        # x' [128,2048] fp32r
        p_xpb = P("xpb", 1)      # x' [128,2048] bf16
        p_x2b = P("x2b", 1)      # x'' [128,2048] bf16 (AG payload)
        p_ht = P("ht", 1)        # [128,8192] bf16: FFN hidden (32 x 256)
        p_wqkv = P("wqkv", 3)    # [128,1024] bf16 (3/layer, tag-shared)
        p_wo = P("wo", 8)        # [128,1024] bf16 (8/layer resident)
        p_w1 = P("w1", 3)        # [128,2048] bf16 stream (2 f-chunks per tile)
        p_w2 = P("w2", 4)        # [128,2048] bf16 stream
        p_bias = P("bias", 2)    # small per-layer bias tiles
        p_stat = P("stat", 4)    # [1,N] stats
        p_bc = P("bc", 4)        # broadcast tiles
        p_sq = P("sq", 3)        # z^2 staging [128,256]
        p_tmp = P("tmp", 4)      # [128,256] fp32r temps
        ps_big = P("ps_big", 2, space="PSUM")  # [128,1024] 2-bank slots (shared tag)
        ps_b = P("ps_b", 4, space="PSUM")    # attV [65,512] / transpose / LN stats
        d_agi = P("d_agi", 2, space="DRAM")
        d_ago = P("d_ago", 2, space="DRAM")
        d_a2i = P("d_a2i", 2, space="DRAM")
        d_a2o = P("d_a2o", 2, space="DRAM")

        # constants
        p_const = ctx.enter_context(tc.tile_pool(name="const", bufs=1))
        ones_f = p_const.tile([128, 1], FP32, name="ones_f", tag="onesf")
        nc.gpsimd.memset(ones_f[:], 1.0)
        ones_sb = p_const.tile([128, 1], FP32R, name="ones_sb", tag="ones")
        nc.scalar.activation(ones_sb[:], ones_f[:], AF.Copy, bias=0.0, scale=1.0)
        eps_sb = p_const.tile([1, 1], FP32, name="eps_sb", tag="eps")
        nc.gpsimd.memset(eps_sb[:], float(EPS))
        ident_sb = p_bias.tile([128, 128], FP32R, name="ident_sb", bufs=1)
        nc.sync.dma_start(ident_sb[:], ident_e[:].bitcast(FP32R))

        # initial x_mine (fp32 residual basis)
        x_mine = p_xst.tile([128, 8 * TOK], FP32R, name="x_mine", tag="xst")
        for dc in range(8):
            nc.sync.dma_start(
                x_mine[:, TOK * dc : TOK * (dc + 1)],
                x0mine_e[128 * dc : 128 * (dc + 1), :].bitcast(FP32R),
            )

        ag_halves = None  # two DRAM tensors [NC, 512, TOK] from the split AG

        ln_env = dict(
            nc=nc, mybir=mybir, ones=ones_sb, eps=eps_sb,
            p_sq=p_sq, p_stat=p_stat, p_bc=p_bc, p_tmp=p_tmp, ps_st=ps_b,
        )

        for l in range(L):
            # -- per-layer weight/bias loads (emitted early: prefetch) --
            wq_sb = p_wqkv.tile([128, 1024], BF, name="wq_sb", tag="wqkv")
            nc.sync.dma_start(wq_sb[:], wq_e[l])
            wk_sb = p_wqkv.tile([128, 1024], BF, name="wk_sb", tag="wqkv")
            nc.sync.dma_start(wk_sb[:], wk_e[l])
            wv_sb = p_wqkv.tile([128, 1024], BF, name="wv_sb", tag="wqkv")
            nc.sync.dma_start(wv_sb[:], wv_e[l])
            bq_sb = p_bias.tile([128, 1], FP32, name="bq_sb", tag="bq")
            nc.sync.dma_start(bq_sb[:], bq_e[l])
            bk_sb = p_bias.tile([128, 1], FP32, name="bk_sb", tag="bk")
            nc.sync.dma_start(bk_sb[:], bk_e[l])
            bv_sb = p_bias.tile([128, 1], FP32, name="bv_sb", tag="bv")
            nc.sync.dma_start(bv_sb[:], bv_e[l])
            wo_sb = []
            for t in range(8):
                w = p_wo.tile([128, 1024], BF, name=f"wo{t}", tag="wo")
                nc.sync.dma_start(w[:], wo_e[l, t])
                wo_sb.append(w)
            bo_sb = p_bias.tile([128, 8], FP32, name="bo_sb", tag="bo")
            nc.sync.dma_start(bo_sb[:], bo_e[l])
            g1_sb = p_bias.tile([128, 8], FP32, name="g1_sb", tag="g1")
            nc.sync.dma_start(g1_sb[:], g1_e[l])
            be1_sb = p_bias.tile([128, 8], FP32, name="be1_sb", tag="be1")
            nc.sync.dma_start(be1_sb[:], be1_e[l])
            b1_sb = p_bias.tile([128, 32], FP32, name="b1_sb", tag="b1")
            nc.sync.dma_start(b1_sb[:], b1_e[l])
            b2_sb = p_bias.tile([128, 8], FP32, name="b2_sb", tag="b2")
            nc.sync.dma_start(b2_sb[:], b2_e[l])
            g2_sb = p_bias.tile([128, 8], FP32, name="g2_sb", tag="g2")
            nc.sync.dma_start(g2_sb[:], g2_e[l])
            be2_sb = p_bias.tile([128, 8], FP32, name="be2_sb", tag="be2")
            nc.sync.dma_start(be2_sb[:], be2_e[l])

            # ---------- QKV ----------
            # bf16 for q/k/exp/vaug: HW runs fp32r matmuls in fp32_mode=HIGH
            # (~4 cyc/row); bf16 is 1 cyc/row and the 2e-2 tolerance allows it.
            qt = p_qk.tile([128, 2048], BF, name="qt", tag="qk")
            kt = p_qk.tile([128, 2048], BF, name="kt", tag="qk")
            vaug = p_vaug.tile([128, 16 * (2 * DK + 2)], BF, name="vaug")
            VW = 2 * DK + 2  # 130: per t-tile block [h0(64)|ones|h1(64)|ones]
            # pre-fill the ones columns once (gpsimd, off the DVE queue)
            nc.gpsimd.memset(vaug[:], 1.0)

            for half in range(2):
                # xg: one token-half (= one batch) of full x^T, 8 d-tiles
                xg = []
                for dt in range(8):
                    t = p_xg.tile([128, 1024], BF, name=f"xg{dt}", tag="xg")
                    if l == 0:
                        nc.sync.dma_start(
                            t[:],
                            x0all_e[
                                128 * dt : 128 * (dt + 1),
                                1024 * half : 1024 * (half + 1),
                            ],
                        )
                    else:
                        # token-half `half` = rank blocks 4*half..4*half+3
                        nc.sync.dma_start(
                            t[:].rearrange("p (r s) -> p r s", r=4),
                            ag_halves[
                                4 * half : 4 * half + 4,
                                128 * dt : 128 * (dt + 1),
                                :,
                            ].rearrange("r p s -> p r s"),
                        )
                    xg.append(t)
                for sc2 in range(2):
                    sc = 2 * half + sc2
                    ssl = slice(512 * sc, 512 * (sc + 1))  # global token chunk
                    hsl = slice(512 * sc2, 512 * (sc2 + 1))  # within-half chunk
                    q_ps = ps_big.tile([128, 512], FP32, name="q_ps", tag="big")
                    for dt in range(8):
                        nc.tensor.matmul(
                            q_ps[:],
                            wq_sb[:, 128 * dt : 128 * (dt + 1)],
                            xg[dt][:, hsl],
                            start=(dt == 0),
                            stop=(dt == 7),
                        )
                    # bias-add on ACT (idle during QKV; frees the DVE queue)
                    nc.scalar.activation(
                        qt[:, ssl], q_ps[:], AF.Identity, bias=bq_sb[:], scale=1.0
                    )
                    k_ps = ps_big.tile([128, 512], FP32, name="k_ps", tag="big")
                    for dt in range(8):
                        nc.tensor.matmul(
                            k_ps[:],
                            wk_sb[:, 128 * dt : 128 * (dt + 1)],
                            xg[dt][:, hsl],
                            start=(dt == 0),
                            stop=(dt == 7),
                        )
                    nc.scalar.activation(
                        kt[:, ssl], k_ps[:], AF.Identity, bias=bk_sb[:], scale=1.0
                    )
                    v_ps = ps_big.tile([128, 512], FP32, name="v_ps", tag="big")
                    for dt in range(8):
                        nc.tensor.matmul(
                            v_ps[:],
                            wv_sb[:, 128 * dt : 128 * (dt + 1)],
                            xg[dt][:, hsl],
                            start=(dt == 0),
                            stop=(dt == 7),
                        )
                    vtT = p_vtT.tile([128, 512], FP32R, name="vtT", tag="vtT")
                    nc.scalar.activation(
                        vtT[:], v_ps[:], AF.Identity, bias=bv_sb[:], scale=1.0
                    )
                    for j in range(4):
                        tt = 4 * sc + j  # global t-tile 0..15
                        tr_ps = ps_b.tile([128, 128], FP32, name="tr_ps", tag="ps")
                        nc.tensor.transpose(
                            tr_ps[:].bitcast(FP32R),
                            vtT[:, 128 * j : 128 * (j + 1)],
                            ident_sb[:],
                        )
                        o = VW * tt
                        nc.vector.tensor_copy(vaug[:, o : o + 64], tr_ps[:, 0:64])
                        nc.vector.tensor_copy(
                            vaug[:, o + 65 : o + 129], tr_ps[:, 64:128]
                        )

            # ---------- attention ----------
            # Two independent 512-token streams (the sc chunks of one batch)
            # interleave so the PE never waits on the scores->exp->attV chain.
            ota = p_ota.tile([128, 2048], BF, name="ota")
            a2i = d_a2i.tile([NC, 128, TOK], BF, name="a2i", tag="a2i")
            for b in range(B):
                base = S * b
                ssls = [
                    slice(base + 512 * sc, base + 512 * (sc + 1)) for sc in range(2)
                ]
                o_ps = {
                    (sc, h): ps_b.tile([65, 512], FP32, name=f"o_ps{sc}{h}", tag="ps")
                    for sc in range(2)
                    for h in range(HPC)
                }
                e_prev = {0: None, 1: None}
                for jt in range(9):
                    for sc in range(2):
                        if jt < 8:
                            tsl = slice(base + 128 * jt, base + 128 * (jt + 1))
                            # both heads' scores into one 2-bank PSUM tile,
                            # then a single [128,1024] exp on ACT
                            s2 = ps_big.tile(
                                [128, 1024], FP32, name="s2", tag="big"
                            )
                            for h in range(HPC):
                                hp = slice(64 * h, 64 * (h + 1))
                                nc.tensor.matmul(
                                    s2[:, 512 * h : 512 * (h + 1)],
                                    kt[hp, tsl],
                                    qt[hp, ssls[sc]],
                                    start=True,
                                    stop=True,
                                )
                            e2 = p_exp.tile(
                                [128, 1024], BF, name="e2", tag="e"
                            )
                            nc.scalar.activation(
                                e2[:], s2[:], AF.Exp, bias=0.0, scale=0.125
                            )
                        if jt > 0:
                            pj = jt - 1
                            for h in range(HPC):
                                o = VW * (8 * b + pj) + (DK + 1) * h
                                nc.tensor.matmul(
                                    o_ps[(sc, h)][:],
                                    vaug[:, o : o + 65],
                                    e_prev[sc][:, 512 * h : 512 * (h + 1)],
                                    start=(pj == 0),
                                    stop=(pj == 7),
                                )
                        e_prev[sc] = e2 if jt < 8 else None
                for sc in range(2):
                    for h in range(HPC):
                        drow = p_stat.tile(
                            [1, 512], FP32, name="drow", tag="drow", bufs=2
                        )
                        nc.vector.tensor_copy(drow[:], o_ps[(sc, h)][64:65, :])
                        den = p_stat.tile(
                            [1, 512], FP32, name="den", tag="den", bufs=2
                        )
                        nc.vector.reciprocal_approx_fast(den[:], drow[:])
                        den_bc = p_bc.tile(
                            [64, 512], FP32, name="den_bc", tag="dbc", bufs=2
                        )
                        nc.gpsimd.partition_broadcast(den_bc[:], den[:])
                        nc.vector.tensor_mul(
                            ota[64 * h : 64 * (h + 1), ssls[sc]],
                            o_ps[(sc, h)][0:64, :],
                            den_bc[:].bitcast(FP32R),
                        )
                    # ship this 512-token quadrant (= 2 rank chunks) to the
                    # A2A bounce as soon as both heads are normalized
                    rr = 2 * (2 * b + sc)
                    nc.sync.dma_start(
                        a2i[rr : rr + 2].rearrange("r p s -> p r s"),
                        ota[:, ssls[sc]].rearrange("p (r s) -> p r s", r=2),
                    )

            # ---------- AllToAll + out-projection + LN1 ----------
            a2o = d_a2o.tile([NC, 128, TOK], BF, name="a2o", tag="a2o")
            nc.gpsimd.collective_compute(
                "AllToAll",
                ALU.bypass,
                replica_groups=RG,
                ins=[a2i[:].opt()],
                outs=[a2o[:].opt()],
            )
            otf = p_otf.tile([128, 8 * TOK], BF, name="otf")
            for t in range(NC):
                nc.sync.dma_start(otf[:, TOK * t : TOK * (t + 1)], a2o[t])

            z1 = p_z.tile([128, 8 * TOK], FP32R, name="z1", tag="z")
            sum1_ps = ps_b.tile([1, TOK], FP32, name="sum1_ps", tag="ps")
            sq1_ps = ps_b.tile([1, TOK], FP32, name="sq1_ps", tag="ps")
            for dc in range(8):
                dsl = slice(TOK * dc, TOK * (dc + 1))
                y_ps = ps_big.tile([128, TOK], FP32, name="y_ps", tag="big")
                for t in range(8):
                    nc.tensor.matmul(
                        y_ps[:],
                        wo_sb[t][:, 128 * dc : 128 * (dc + 1)],
                        otf[:, TOK * t : TOK * (t + 1)],
                        start=(t == 0),
                        stop=(t == 7),
                    )
                nc.vector.scalar_tensor_tensor(
                    z1[:, dsl], y_ps[:], bo_sb[:, dc : dc + 1], x_mine[:, dsl],
                    ALU.add, ALU.add,
                )
                # interleaved LN1 statistics
                nc.tensor.matmul(
                    sum1_ps[:], ones_sb[:], z1[:, dsl],
                    start=(dc == 0), stop=(dc == 7),
                )
                zsq = p_sq.tile([128, TOK], FP32R, name="zsq", tag="sq")
                nc.scalar.square(zsq[:], z1[:, dsl])
                nc.tensor.matmul(
                    sq1_ps[:], ones_sb[:], zsq[:],
                    start=(dc == 0), stop=(dc == 7),
                )

            xp = p_xp.tile([128, 8 * TOK], FP32R, name="xp")
            xpb = p_xpb.tile([128, 8 * TOK], BF, name="xpb")
            _emit_layernorm(ln_env, z1, g1_sb, be1_sb, xp, xpb, sum1_ps, sq1_ps)

            # ---------- FFN + LN2 (+ split AllGather for next layer) ----------
            ht = p_ht.tile([128, 32 * TOK], BF, name="ht")
            for g in range(16):
                w1t = p_w1.tile([128, 2048], BF, name="w1t", tag="w1")
                nc.sync.dma_start(
                    w1t[:].rearrange("p (c j) -> p c j", c=2),
                    w1_e[l, 2 * g : 2 * g + 2].rearrange("c p j -> p c j"),
                )
                for c in range(2):
                    fc = 2 * g + c
                    h_ps = ps_big.tile([128, TOK], FP32, name="h_ps", tag="big")
                    for dt in range(8):
                        nc.tensor.matmul(
                            h_ps[:],
                            w1t[:, 1024 * c + 128 * dt : 1024 * c + 128 * (dt + 1)],
                            xpb[:, TOK * dt : TOK * (dt + 1)],
                            start=(dt == 0),
                            stop=(dt == 7),
                        )
                    # relu(h + b1) on DVE (keeps ACT free for attention exp)
                    nc.vector.tensor_scalar(
                        ht[:, TOK * fc : TOK * (fc + 1)],
                        h_ps[:],
                        b1_sb[:, fc : fc + 1],
                        0.0,
                        ALU.add,
                        ALU.max,
                    )

            z2 = p_z.tile([128, 8 * TOK], FP32R, name="z2", tag="z")
            sum2_ps = ps_b.tile([1, TOK], FP32, name="sum2_ps", tag="ps")
            sq2_ps = ps_b.tile([1, TOK], FP32, name="sq2_ps", tag="ps")
            for dc in range(8):
                dsl = slice(TOK * dc, TOK * (dc + 1))
                y2_ps = ps_big.tile([128, TOK], FP32, name="y2_ps", tag="big")
                for half in range(2):
                    w2t = p_w2.tile([128, 2048], BF, name="w2t", tag="w2")
                    nc.sync.dma_start(w2t[:], w2_e[l, dc, half])
                    for ft in range(16):
                        gt = 16 * half + ft
                        nc.tensor.matmul(
                            y2_ps[:],
                            w2t[:, 128 * ft : 128 * (ft + 1)],
                            ht[:, TOK * gt : TOK * (gt + 1)],
                            start=(gt == 0),
                            stop=(gt == 31),
                        )
                nc.vector.scalar_tensor_tensor(
                    z2[:, dsl], y2_ps[:], b2_sb[:, dc : dc + 1], xp[:, dsl],
                    ALU.add, ALU.add,
                )
                # interleaved LN2 statistics
                nc.tensor.matmul(
                    sum2_ps[:], ones_sb[:], z2[:, dsl],
                    start=(dc == 0), stop=(dc == 7),
                )
                zsq2 = p_sq.tile([128, TOK], FP32R, name="zsq2", tag="sq")
                nc.scalar.square(zsq2[:], z2[:, dsl])
                nc.tensor.matmul(
                    sq2_ps[:], ones_sb[:], zsq2[:],
                    start=(dc == 0), stop=(dc == 7),
                )

            x_mine = p_xst.tile([128, 8 * TOK], FP32R, name="x_mine", tag="xst")
            if l < L - 1:
                x2b = p_x2b.tile([128, 8 * TOK], BF, name="x2b")
                agi = d_agi.tile([D, TOK], BF, name="agi", tag="agi")
                ago = d_ago.tile(
                    [NC, D, TOK], BF, name="ago", tag="ago", addr_space="Shared"
                )

                def post_dc(dc, _x2b=x2b, _agi=agi, _ago=ago):
                    # ship each normalized d-chunk to the AG bounce as it
                    # completes; fire the single AllGather after the last
                    # (collectives have ~11us fixed cost - do NOT split them)
                    nc.sync.dma_start(
                        _agi[128 * dc : 128 * (dc + 1), :],
                        _x2b[:, TOK * dc : TOK * (dc + 1)],
                    )
                    if dc == 7:
                        nc.gpsimd.collective_compute(
                            "AllGather",
                            ALU.bypass,
                            replica_groups=RG,
                            ins=[_agi[:].opt()],
                            outs=[_ago[:].opt()],
                        )

                _emit_layernorm(
                    ln_env, z2, g2_sb, be2_sb, x_mine, x2b, sum2_ps, sq2_ps,
                    post_dc=post_dc,
                )
                ag_halves = ago
            else:
                _emit_layernorm(ln_env, z2, g2_sb, be2_sb, x_mine, None, sum2_ps, sq2_ps)
                for dc in range(8):
                    nc.sync.dma_start(
                        out_e[128 * dc : 128 * (dc + 1), :].bitcast(FP32R),
                        x_mine[:, TOK * dc : TOK * (dc + 1)],
                    )

    nc.compile()
    return nc


def _emit_layernorm(env, z, g_sb, b_sb, out_fp, out_bf, sum_ps, sq_ps,
                    post_dc=None):
    """LayerNorm over D; sum/sumsq PSUM stats are pre-accumulated by caller."""
    nc = env["nc"]
    mybir = env["mybir"]
    FP32 = mybir.dt.float32
    FP32R = mybir.dt.float32r
    AF = mybir.ActivationFunctionType
    ALU = mybir.AluOpType
    eps_sb = env["eps"]
    p_stat, p_bc, p_tmp = env["p_stat"], env["p_bc"], env["p_tmp"]

    st = lambda nm: p_stat.tile([1, TOK], FP32, name=nm, tag="lnstat", bufs=8)
    mu = st("mu")
    nc.vector.tensor_scalar_mul(mu[:], sum_ps[:], 1.0 / D)
    ex2 = st("ex2")
    nc.vector.tensor_scalar_mul(ex2[:], sq_ps[:], 1.0 / D)
    var = st("var")
    musq = st("musq")
    nc.vector.tensor_mul(musq[:], mu[:], mu[:])
    nc.vector.tensor_sub(var[:], ex2[:], musq[:])
    std = st("std")
    nc.scalar.activation(std[:], var[:], AF.Sqrt, bias=eps_sb[:], scale=1.0)
    rsig = st("rsig")
    nc.vector.reciprocal_approx_fast(rsig[:], std[:])
    mu_bc = p_bc.tile([128, TOK], FP32, name="mu_bc", tag="mu_bc", bufs=2)
    nc.gpsimd.partition_broadcast(mu_bc[:], mu[:])
    rs_bc = p_bc.tile([128, TOK], FP32, name="rs_bc", tag="rs_bc", bufs=2)
    nc.gpsimd.partition_broadcast(rs_bc[:], rsig[:])
    AF = mybir.ActivationFunctionType
    for dc in range(8):
        dsl = slice(TOK * dc, TOK * (dc + 1))
        t1 = p_tmp.tile([128, TOK], FP32R, name="t1", tag="tmp")
        nc.vector.tensor_sub(t1[:], z[:, dsl], mu_bc[:].bitcast(FP32R))
        t2 = p_tmp.tile([128, TOK], FP32R, name="t2", tag="tmp")
        nc.vector.tensor_mul(t2[:], t1[:], rs_bc[:].bitcast(FP32R))
        # gamma*x + beta on ACT (idle here) - halves the DVE serial tail
        nc.scalar.activation(
            out_fp[:, dsl],
            t2[:],
            AF.Identity,
            bias=b_sb[:, dc : dc + 1],
            scale=g_sb[:, dc : dc + 1],
        )
        if out_bf is not None:
            nc.vector.tensor_copy(out_bf[:, dsl], out_fp[:, dsl])
        if post_dc is not None:
            post_dc(dc)


def _pack_inputs(src, Wq, bq, Wk, bk, Wv, bv, Wo, bo, ln1_g, ln1_b,
                 W1, b1, W2, b2, ln2_g, ln2_b):
    """Host-side sharding/packing. Returns list of per-core input dicts."""
    f32 = np.float32
    # positional encoding (phase == pos, since floor(dim/D) == 0)
    pos = np.arange(S, dtype=f32).reshape(-1, 1)
    dim = np.arange(D)
    pe = np.where(dim[None, :] % 2 == 0, np.sin(pos), np.cos(pos)).astype(f32)
    x0 = (np.asarray(src, f32) + pe[None]).reshape(B * S, D)
    x0T = np.ascontiguousarray(x0.T)                      # [D, 2048]
    x0T_bf = x0T.astype(NPBF16)

    bf = lambda a: np.ascontiguousarray(a).astype(NPBF16)
    fc = lambda a: np.ascontiguousarray(a).astype(f32)

    Wo_p = bf(np.asarray(Wo, f32).reshape(L, 8, 128, 1024))
    bo_p = fc(np.asarray(bo, f32).reshape(L, 8, 128).transpose(0, 2, 1))
    g1_p = fc(np.asarray(ln1_g, f32).reshape(L, 8, 128).transpose(0, 2, 1))
    be1_p = fc(np.asarray(ln1_b, f32).reshape(L, 8, 128).transpose(0, 2, 1))
    g2_p = fc(np.asarray(ln2_g, f32).reshape(L, 8, 128).transpose(0, 2, 1))
    be2_p = fc(np.asarray(ln2_b, f32).reshape(L, 8, 128).transpose(0, 2, 1))
    b1_p = fc(np.asarray(b1, f32).reshape(L, 32, 128).transpose(0, 2, 1))
    b2_p = fc(np.asarray(b2, f32).reshape(L, 8, 128).transpose(0, 2, 1))
    # W1: [L, D, F] -> [L, fc, p, (dt j)]
    W1_p = bf(
        np.asarray(W1, f32)
        .reshape(L, 8, 128, 32, 128)
        .transpose(0, 3, 2, 1, 4)
        .reshape(L, 32, 128, 1024)
    )
    # W2: [L, F, D] -> [L, dc, half, p, (ft j)]
    W2_p = bf(
        np.asarray(W2, f32)
        .reshape(L, 32, 128, 8, 128)
        .transpose(0, 3, 1, 2, 4)      # [L, dc, ft, p, j]
        .reshape(L, 8, 2, 16, 128, 128)
        .transpose(0, 1, 2, 4, 3, 5)   # [L, dc, half, p, ft, j]
        .reshape(L, 8, 2, 128, 2048)
    )
    ident = np.eye(128, dtype=f32)

    Wq = np.asarray(Wq, f32)
    Wk = np.asarray(Wk, f32)
    Wv = np.asarray(Wv, f32)
    bq = np.asarray(bq, f32)
    bk = np.asarray(bk, f32)
    bv = np.asarray(bv, f32)

    def pack_headw(Wx, r):
        # [L, D, 128] for heads 2r, 2r+1 -> [L, 128, (dt j)]
        cat = np.concatenate([Wx[:, 2 * r], Wx[:, 2 * r + 1]], axis=2)  # [L,D,128]
        return bf(
            cat.reshape(L, 8, 128, 128).transpose(0, 2, 1, 3).reshape(L, 128, 1024)
        )

    in_maps = []
    for r in range(NC):
        m = {
            "x0all": x0T_bf,
            "x0mine": np.ascontiguousarray(x0T[:, TOK * r : TOK * (r + 1)]),
            "wq": pack_headw(Wq, r),
            "wk": pack_headw(Wk, r),
            "wv": pack_headw(Wv, r),
            "bq": fc(np.concatenate([bq[:, 2 * r], bq[:, 2 * r + 1]], axis=1))[
                :, :, None
            ],
            "bk": fc(np.concatenate([bk[:, 2 * r], bk[:, 2 * r + 1]], axis=1))[
                :, :, None
            ],
            "bv": fc(np.concatenate([bv[:, 2 * r], bv[:, 2 * r + 1]], axis=1))[
                :, :, None
            ],
            "wo": Wo_p,
            "bo": bo_p,
            "w1": W1_p,
            "b1": b1_p,
            "w2": W2_p,
            "b2": b2_p,
            "g1": g1_p,
            "be1": be1_p,
            "g2": g2_p,
            "be2": be2_p,
            "ident": ident,
        }
        in_maps.append(m)
    return in_maps


def run(inputs, trace=False, trace_kwargs=None):
    """Build (cached), execute on 8 cores, return (output, BassKernelResults)."""
    from concourse.bass_utils import run_bass_kernel_spmd

    if "prog" not in _CACHE:
        _CACHE["prog"] = _build_program()
    nc = _CACHE["prog"]
    in_maps = _pack_inputs(**inputs)
    res = run_bass_kernel_spmd(
        nc, in_maps, list(range(NC)), trace=trace, **(trace_kwargs or {})
    )
    xT = np.empty((B * S, D), np.float32)
    for r in range(NC):
        xT[TOK * r : TOK * (r + 1)] = res.results[r]["out_xT"].T
    return xT.reshape(B, S, D), res


def kernel(**inputs):
    out, _ = run(inputs, trace=False)
    return out



# revision 20
# speedup vs baseline: 1.0744x; 1.0008x over previous
"""Trainium2 Bass kernel for a 6-layer transformer encoder (nn_Encoder).

Sharding across 8 NeuronCores:
  - Attention is tensor-parallel over heads: core r owns heads 2r, 2r+1 and
    computes Q/K/V + scores + softmax + att@V for those heads over ALL tokens.
  - Everything row-wise (out-projection, residuals, LayerNorms, FFN) is
    token-parallel: core r owns tokens [256r, 256r+256) of the flattened
    (batch*seq) axis, with the full Wo/W1/W2 replicated.
  - Per layer only two cheap collectives are needed:
      AllGather of the layer input x^T (bf16, split in two halves so the
      second half overlaps the first half's QKV matmuls)
      AllToAll of per-head attention outputs   -> head-sharded -> token-sharded
    (No AllReduce anywhere: contractions over sharded axes are avoided by
    construction.)

Activations live transposed ("T-layout", [feature, token]) so every matmul
consumes natural-layout weights as the stationary operand. LayerNorm
reductions over D (the partition axis) use ones-matmuls on the PE;
mean/rstd are broadcast back across partitions with gpsimd.partition_broadcast.
Softmax skips max-subtraction (scores are bounded, |S|<6) and its denominator
is computed for free by appending a ones-column to V in the att@V matmul.

dtypes: bf16 for the big GEMMs (QKV / out-proj / FFN weights and moving
operands; fp32 PSUM accumulate), float32r (full-speed fp32, ~1e-4) for
scores / att@V / LN statistics.
"""
import numpy as np
import ml_dtypes

L, H, D, DK, F = 6, 16, 1024, 64, 4096
B, S = 2, 1024
NC = 8
TOK = (B * S) // NC  # 256 tokens per core
HPC = H // NC        # 2 heads per core
EPS = 1e-5
NPBF16 = ml_dtypes.bfloat16

_CACHE = {}


def _build_program():
    import concourse.bacc as bacc
    import concourse.tile as tile
    import concourse.mybir as mybir
    from contextlib import ExitStack

    FP32 = mybir.dt.float32
    FP32R = mybir.dt.float32r
    BF = mybir.dt.bfloat16
    AF = mybir.ActivationFunctionType
    ALU = mybir.AluOpType

    nc = bacc.Bacc(
        "TRN2",
        target_bir_lowering=False,
        debug=False,
        enable_asserts=False,
        num_devices=NC,
    )

    # ---------------- external I/O ----------------
    x0all_e = nc.dram_tensor("x0all", [D, B * S], BF, kind="ExternalInput")
    x0mine_e = nc.dram_tensor("x0mine", [D, TOK], FP32, kind="ExternalInput")
    wq_e = nc.dram_tensor("wq", [L, 128, 1024], BF, kind="ExternalInput")
    wk_e = nc.dram_tensor("wk", [L, 128, 1024], BF, kind="ExternalInput")
    wv_e = nc.dram_tensor("wv", [L, 128, 1024], BF, kind="ExternalInput")
    bq_e = nc.dram_tensor("bq", [L, 128, 1], FP32, kind="ExternalInput")
    bk_e = nc.dram_tensor("bk", [L, 128, 1], FP32, kind="ExternalInput")
    bv_e = nc.dram_tensor("bv", [L, 128, 1], FP32, kind="ExternalInput")
    wo_e = nc.dram_tensor("wo", [L, 8, 128, 1024], BF, kind="ExternalInput")
    bo_e = nc.dram_tensor("bo", [L, 128, 8], FP32, kind="ExternalInput")
    w1_e = nc.dram_tensor("w1", [L, 32, 128, 1024], BF, kind="ExternalInput")
    b1_e = nc.dram_tensor("b1", [L, 128, 32], FP32, kind="ExternalInput")
    w2_e = nc.dram_tensor("w2", [L, 8, 2, 128, 2048], BF, kind="ExternalInput")
    b2_e = nc.dram_tensor("b2", [L, 128, 8], FP32, kind="ExternalInput")
    g1_e = nc.dram_tensor("g1", [L, 128, 8], FP32, kind="ExternalInput")
    be1_e = nc.dram_tensor("be1", [L, 128, 8], FP32, kind="ExternalInput")
    g2_e = nc.dram_tensor("g2", [L, 128, 8], FP32, kind="ExternalInput")
    be2_e = nc.dram_tensor("be2", [L, 128, 8], FP32, kind="ExternalInput")
    ident_e = nc.dram_tensor("ident", [128, 128], FP32, kind="ExternalInput")
    out_e = nc.dram_tensor("out_xT", [D, TOK], FP32, kind="ExternalOutput")

    RG = [list(range(NC))]

    with tile.TileContext(nc) as tc, ExitStack() as ctx:
        P = lambda name, bufs, **kw: ctx.enter_context(
            tc.tile_pool(name=name, bufs=bufs, **kw)
        )
        p_xg = P("xg", 8)        # [128,1024] bf16: x^T d-tiles (one token-half)
        p_qk = P("qk", 2)        # qt/kt [128,2048] fp32r
        p_vtT = P("vtT", 2)      # [128,512] fp32r v^T staging
        p_vaug = P("vaug", 1)    # [128,2080] fp32r: v-natural + ones cols
        p_exp = P("exp", 4)      # [128,1024] fp32r exp(scores^T), 2 heads packed
        p_ota = P("ota", 1)      # [128,2048] bf16: o^T (my heads, all tokens)
        p_otf = P("otf", 1)      # [128,2048] bf16: o^T (all heads, my tokens)
        p_xst = P("xst", 2)      # x_mine [128,2048] fp32r (rotates per layer)
        p_z = P("z", 1)          # residual sums [128,2048] fp32r
        p_xp = P("xp", 1)        # x' [128,2048] fp32r
        p_xpb = P("xpb", 1)      # x' [128,2048] bf16
        p_x2b = P("x2b", 1)      # x'' [128,2048] bf16 (AG payload)
        p_ht = P("ht", 1)        # [128,8192] bf16: FFN hidden (32 x 256)
        p_wqkv = P("wqkv", 3)    # [128,1024] bf16 (3/layer, tag-shared)
        p_wo = P("wo", 8)        # [128,1024] bf16 (8/layer resident)
        p_w1 = P("w1", 3)        # [128,2048] bf16 stream (2 f-chunks per tile)
        p_w2 = P("w2", 4)        # [128,2048] bf16 stream
        p_bias = P("bias", 2)    # small per-layer bias tiles
        p_stat = P("stat", 4)    # [1,N] stats
        p_bc = P("bc", 4)        # broadcast tiles
        p_sq = P("sq", 3)        # z^2 staging [128,256]
        p_tmp = P("tmp", 4)      # [128,256] fp32r temps
        ps_big = P("ps_big", 2, space="PSUM")  # [128,1024] 2-bank slots (shared tag)
        ps_b = P("ps_b", 4, space="PSUM")    # attV [65,512] / transpose / LN stats
        d_agi = P("d_agi", 3, space="DRAM")
        d_ago = P("d_ago", 3, space="DRAM")
        d_a2i = P("d_a2i", 3, space="DRAM")
        d_a2o = P("d_a2o", 3, space="DRAM")

        # constants
        p_const = ctx.enter_context(tc.tile_pool(name="const", bufs=1))
        ones_f = p_const.tile([128, 1], FP32, name="ones_f", tag="onesf")
        nc.gpsimd.memset(ones_f[:], 1.0)
        ones_sb = p_const.tile([128, 1], FP32R, name="ones_sb", tag="ones")
        nc.scalar.activation(ones_sb[:], ones_f[:], AF.Copy, bias=0.0, scale=1.0)
        eps_sb = p_const.tile([1, 1], FP32, name="eps_sb", tag="eps")
        nc.gpsimd.memset(eps_sb[:], float(EPS))
        ident_sb = p_bias.tile([128, 128], FP32R, name="ident_sb", bufs=1)
        nc.sync.dma_start(ident_sb[:], ident_e[:].bitcast(FP32R))

        # warmup collectives: absorb rank-start skew / cold comm-channel cost
        # (~50us each on the first real collectives otherwise) while layer-0
        # QKV+attention compute runs
        wu_i = d_agi.tile([128, 8], BF, name="wu_i", tag="wu_i")
        wu_o = d_ago.tile(
            [NC, 128, 8], BF, name="wu_o", tag="wu_o", addr_space="Shared"
        )
        nc.gpsimd.collective_compute(
            "AllGather", mybir.AluOpType.bypass, replica_groups=RG,
            ins=[wu_i[:].opt()], outs=[wu_o[:].opt()],
        )
        wu2_i = d_a2i.tile([NC, 128, 1], BF, name="wu2_i", tag="wu2_i")
        wu2_o = d_a2o.tile([NC, 128, 1], BF, name="wu2_o", tag="wu2_o")
        nc.gpsimd.collective_compute(
            "AllToAll", mybir.AluOpType.bypass, replica_groups=RG,
            ins=[wu2_i[:].opt()], outs=[wu2_o[:].opt()],
        )

        # initial x_mine (fp32 residual basis)
        x_mine = p_xst.tile([128, 8 * TOK], FP32R, name="x_mine", tag="xst")
        for dc in range(8):
            nc.sync.dma_start(
                x_mine[:, TOK * dc : TOK * (dc + 1)],
                x0mine_e[128 * dc : 128 * (dc + 1), :].bitcast(FP32R),
            )

        ag_halves = None  # two DRAM tensors [NC, 512, TOK] from the split AG

        ln_env = dict(
            nc=nc, mybir=mybir, ones=ones_sb, eps=eps_sb,
            p_sq=p_sq, p_stat=p_stat, p_bc=p_bc, p_tmp=p_tmp, ps_st=ps_b,
        )

        for l in range(L):
            # -- per-layer weight/bias loads (emitted early: prefetch) --
            wq_sb = p_wqkv.tile([128, 1024], BF, name="wq_sb", tag="wqkv")
            nc.sync.dma_start(wq_sb[:], wq_e[l])
            wk_sb = p_wqkv.tile([128, 1024], BF, name="wk_sb", tag="wqkv")
            nc.sync.dma_start(wk_sb[:], wk_e[l])
            wv_sb = p_wqkv.tile([128, 1024], BF, name="wv_sb", tag="wqkv")
            nc.sync.dma_start(wv_sb[:], wv_e[l])
            bq_sb = p_bias.tile([128, 1], FP32, name="bq_sb", tag="bq")
            nc.sync.dma_start(bq_sb[:], bq_e[l])
            bk_sb = p_bias.tile([128, 1], FP32, name="bk_sb", tag="bk")
            nc.sync.dma_start(bk_sb[:], bk_e[l])
            bv_sb = p_bias.tile([128, 1], FP32, name="bv_sb", tag="bv")
            nc.sync.dma_start(bv_sb[:], bv_e[l])
            wo_sb = []
            for t in range(8):
                w = p_wo.tile([128, 1024], BF, name=f"wo{t}", tag="wo")
                nc.sync.dma_start(w[:], wo_e[l, t])
                wo_sb.append(w)
            bo_sb = p_bias.tile([128, 8], FP32, name="bo_sb", tag="bo")
            nc.sync.dma_start(bo_sb[:], bo_e[l])
            g1_sb = p_bias.tile([128, 8], FP32, name="g1_sb", tag="g1")
            nc.sync.dma_start(g1_sb[:], g1_e[l])
            be1_sb = p_bias.tile([128, 8], FP32, name="be1_sb", tag="be1")
            nc.sync.dma_start(be1_sb[:], be1_e[l])
            b1_sb = p_bias.tile([128, 32], FP32, name="b1_sb", tag="b1")
            nc.sync.dma_start(b1_sb[:], b1_e[l])
            b2_sb = p_bias.tile([128, 8], FP32, name="b2_sb", tag="b2")
            nc.sync.dma_start(b2_sb[:], b2_e[l])
            g2_sb = p_bias.tile([128, 8], FP32, name="g2_sb", tag="g2")
            nc.sync.dma_start(g2_sb[:], g2_e[l])
            be2_sb = p_bias.tile([128, 8], FP32, name="be2_sb", tag="be2")
            nc.sync.dma_start(be2_sb[:], be2_e[l])

            # ---------- QKV ----------
            # bf16 for q/k/exp/vaug: HW runs fp32r matmuls in fp32_mode=HIGH
            # (~4 cyc/row); bf16 is 1 cyc/row and the 2e-2 tolerance allows it.
            qt = p_qk.tile([128, 2048], BF, name="qt", tag="qk")
            kt = p_qk.tile([128, 2048], BF, name="kt", tag="qk")
            vaug = p_vaug.tile([128, 16 * (2 * DK + 2)], BF, name="vaug")
            VW = 2 * DK + 2  # 130: per t-tile block [h0(64)|ones|h1(64)|ones]
            # pre-fill the ones columns once (gpsimd, off the DVE queue)
            nc.gpsimd.memset(vaug[:], 1.0)

            for half in range(2):
                # xg: one token-half (= one batch) of full x^T, 8 d-tiles
                xg = []
                for dt in range(8):
                    t = p_xg.tile([128, 1024], BF, name=f"xg{dt}", tag="xg")
                    if l == 0:
                        nc.sync.dma_start(
                            t[:],
                            x0all_e[
                                128 * dt : 128 * (dt + 1),
                                1024 * half : 1024 * (half + 1),
                            ],
                        )
                    else:
                        # token-half `half` = rank blocks 4*half..4*half+3
                        nc.sync.dma_start(
                            t[:].rearrange("p (r s) -> p r s", r=4),
                            ag_halves[
                                4 * half : 4 * half + 4,
                                128 * dt : 128 * (dt + 1),
                                :,
                            ].rearrange("r p s -> p r s"),
                        )
                    xg.append(t)
                for sc2 in range(2):
                    sc = 2 * half + sc2
                    ssl = slice(512 * sc, 512 * (sc + 1))  # global token chunk
                    hsl = slice(512 * sc2, 512 * (sc2 + 1))  # within-half chunk
                    q_ps = ps_big.tile([128, 512], FP32, name="q_ps", tag="big")
                    for dt in range(8):
                        nc.tensor.matmul(
                            q_ps[:],
                            wq_sb[:, 128 * dt : 128 * (dt + 1)],
                            xg[dt][:, hsl],
                            start=(dt == 0),
                            stop=(dt == 7),
                        )
                    # bias-add on ACT (idle during QKV; frees the DVE queue)
                    nc.scalar.activation(
                        qt[:, ssl], q_ps[:], AF.Identity, bias=bq_sb[:], scale=1.0
                    )
                    k_ps = ps_big.tile([128, 512], FP32, name="k_ps", tag="big")
                    for dt in range(8):
                        nc.tensor.matmul(
                            k_ps[:],
                            wk_sb[:, 128 * dt : 128 * (dt + 1)],
                            xg[dt][:, hsl],
                            start=(dt == 0),
                            stop=(dt == 7),
                        )
                    nc.scalar.activation(
                        kt[:, ssl], k_ps[:], AF.Identity, bias=bk_sb[:], scale=1.0
                    )
                    v_ps = ps_big.tile([128, 512], FP32, name="v_ps", tag="big")
                    for dt in range(8):
                        nc.tensor.matmul(
                            v_ps[:],
                            wv_sb[:, 128 * dt : 128 * (dt + 1)],
                            xg[dt][:, hsl],
                            start=(dt == 0),
                            stop=(dt == 7),
                        )
                    vtT = p_vtT.tile([128, 512], FP32R, name="vtT", tag="vtT")
                    nc.scalar.activation(
                        vtT[:], v_ps[:], AF.Identity, bias=bv_sb[:], scale=1.0
                    )
                    for j in range(4):
                        tt = 4 * sc + j  # global t-tile 0..15
                        tr_ps = ps_b.tile([128, 128], FP32, name="tr_ps", tag="ps")
                        nc.tensor.transpose(
                            tr_ps[:].bitcast(FP32R),
                            vtT[:, 128 * j : 128 * (j + 1)],
                            ident_sb[:],
                        )
                        o = VW * tt
                        nc.vector.tensor_copy(vaug[:, o : o + 64], tr_ps[:, 0:64])
                        nc.vector.tensor_copy(
                            vaug[:, o + 65 : o + 129], tr_ps[:, 64:128]
                        )

            # ---------- attention ----------
            # Two independent 512-token streams (the sc chunks of one batch)
            # interleave so the PE never waits on the scores->exp->attV chain.
            ota = p_ota.tile([128, 2048], BF, name="ota")
            a2i = d_a2i.tile([NC, 128, TOK], BF, name="a2i", tag="a2i")
            for b in range(B):
                base = S * b
                ssls = [
                    slice(base + 512 * sc, base + 512 * (sc + 1)) for sc in range(2)
                ]
                o_ps = {
                    (sc, h): ps_b.tile([65, 512], FP32, name=f"o_ps{sc}{h}", tag="ps")
                    for sc in range(2)
                    for h in range(HPC)
                }
                e_prev = {0: None, 1: None}
                for jt in range(9):
                    for sc in range(2):
                        if jt < 8:
                            tsl = slice(base + 128 * jt, base + 128 * (jt + 1))
                            # both heads' scores into one 2-bank PSUM tile,
                            # then a single [128,1024] exp on ACT
                            s2 = ps_big.tile(
                                [128, 1024], FP32, name="s2", tag="big"
                            )
                            for h in range(HPC):
                                hp = slice(64 * h, 64 * (h + 1))
                                nc.tensor.matmul(
                                    s2[:, 512 * h : 512 * (h + 1)],
                                    kt[hp, tsl],
                                    qt[hp, ssls[sc]],
                                    start=True,
                                    stop=True,
                                )
                            e2 = p_exp.tile(
                                [128, 1024], BF, name="e2", tag="e"
                            )
                            nc.scalar.activation(
                                e2[:], s2[:], AF.Exp, bias=0.0, scale=0.125
                            )
                        if jt > 0:
                            pj = jt - 1
                            for h in range(HPC):
                                o = VW * (8 * b + pj) + (DK + 1) * h
                                nc.tensor.matmul(
                                    o_ps[(sc, h)][:],
                                    vaug[:, o : o + 65],
                                    e_prev[sc][:, 512 * h : 512 * (h + 1)],
                                    start=(pj == 0),
                                    stop=(pj == 7),
                                )
                        e_prev[sc] = e2 if jt < 8 else None
                for sc in range(2):
                    for h in range(HPC):
                        drow = p_stat.tile(
                            [1, 512], FP32, name="drow", tag="drow", bufs=2
                        )
                        nc.vector.tensor_copy(drow[:], o_ps[(sc, h)][64:65, :])
                        den = p_stat.tile(
                            [1, 512], FP32, name="den", tag="den", bufs=2
                        )
                        nc.vector.reciprocal_approx_fast(den[:], drow[:])
                        den_bc = p_bc.tile(
                            [64, 512], FP32, name="den_bc", tag="dbc", bufs=2
                        )
                        nc.gpsimd.partition_broadcast(den_bc[:], den[:])
                        nc.vector.tensor_mul(
                            ota[64 * h : 64 * (h + 1), ssls[sc]],
                            o_ps[(sc, h)][0:64, :],
                            den_bc[:].bitcast(FP32R),
                        )
                    # ship this 512-token quadrant (= 2 rank chunks) to the
                    # A2A bounce as soon as both heads are normalized
                    rr = 2 * (2 * b + sc)
                    nc.sync.dma_start(
                        a2i[rr : rr + 2].rearrange("r p s -> p r s"),
                        ota[:, ssls[sc]].rearrange("p (r s) -> p r s", r=2),
                    )

            # ---------- AllToAll + out-projection + LN1 ----------
            a2o = d_a2o.tile([NC, 128, TOK], BF, name="a2o", tag="a2o")
            nc.gpsimd.collective_compute(
                "AllToAll",
                ALU.bypass,
                replica_groups=RG,
                ins=[a2i[:].opt()],
                outs=[a2o[:].opt()],
            )
            otf = p_otf.tile([128, 8 * TOK], BF, name="otf")
            for t in range(NC):
                nc.sync.dma_start(otf[:, TOK * t : TOK * (t + 1)], a2o[t])

            z1 = p_z.tile([128, 8 * TOK], FP32R, name="z1", tag="z")
            sum1_ps = ps_b.tile([1, TOK], FP32, name="sum1_ps", tag="ps")
            sq1_ps = ps_b.tile([1, TOK], FP32, name="sq1_ps", tag="ps")
            for dc in range(8):
                dsl = slice(TOK * dc, TOK * (dc + 1))
                y_ps = ps_big.tile([128, TOK], FP32, name="y_ps", tag="big")
                for t in range(8):
                    nc.tensor.matmul(
                        y_ps[:],
                        wo_sb[t][:, 128 * dc : 128 * (dc + 1)],
                        otf[:, TOK * t : TOK * (t + 1)],
                        start=(t == 0),
                        stop=(t == 7),
                    )
                nc.vector.scalar_tensor_tensor(
                    z1[:, dsl], y_ps[:], bo_sb[:, dc : dc + 1], x_mine[:, dsl],
                    ALU.add, ALU.add,
                )
                # interleaved LN1 statistics
                nc.tensor.matmul(
                    sum1_ps[:], ones_sb[:], z1[:, dsl],
                    start=(dc == 0), stop=(dc == 7),
                )
                zsq = p_sq.tile([128, TOK], FP32R, name="zsq", tag="sq")
                nc.scalar.square(zsq[:], z1[:, dsl])
                nc.tensor.matmul(
                    sq1_ps[:], ones_sb[:], zsq[:],
                    start=(dc == 0), stop=(dc == 7),
                )

            xp = p_xp.tile([128, 8 * TOK], FP32R, name="xp")
            xpb = p_xpb.tile([128, 8 * TOK], BF, name="xpb")
            _emit_layernorm(ln_env, z1, g1_sb, be1_sb, xp, xpb, sum1_ps, sq1_ps)

            # ---------- FFN + LN2 (+ split AllGather for next layer) ----------
            ht = p_ht.tile([128, 32 * TOK], BF, name="ht")
            for g in range(16):
                w1t = p_w1.tile([128, 2048], BF, name="w1t", tag="w1")
                nc.sync.dma_start(
                    w1t[:].rearrange("p (c j) -> p c j", c=2),
                    w1_e[l, 2 * g : 2 * g + 2].rearrange("c p j -> p c j"),
                )
                for c in range(2):
                    fc = 2 * g + c
                    h_ps = ps_big.tile([128, TOK], FP32, name="h_ps", tag="big")
                    for dt in range(8):
                        nc.tensor.matmul(
                            h_ps[:],
                            w1t[:, 1024 * c + 128 * dt : 1024 * c + 128 * (dt + 1)],
                            xpb[:, TOK * dt : TOK * (dt + 1)],
                            start=(dt == 0),
                            stop=(dt == 7),
                        )
                    # relu(h + b1) on DVE (keeps ACT free for attention exp)
                    nc.vector.tensor_scalar(
                        ht[:, TOK * fc : TOK * (fc + 1)],
                        h_ps[:],
                        b1_sb[:, fc : fc + 1],
                        0.0,
                        ALU.add,
                        ALU.max,
                    )

            z2 = p_z.tile([128, 8 * TOK], FP32R, name="z2", tag="z")
            sum2_ps = ps_b.tile([1, TOK], FP32, name="sum2_ps", tag="ps")
            sq2_ps = ps_b.tile([1, TOK], FP32, name="sq2_ps", tag="ps")
            for dc in range(8):
                dsl = slice(TOK * dc, TOK * (dc + 1))
                y2_ps = ps_big.tile([128, TOK], FP32, name="y2_ps", tag="big")
                for half in range(2):
                    w2t = p_w2.tile([128, 2048], BF, name="w2t", tag="w2")
                    nc.sync.dma_start(w2t[:], w2_e[l, dc, half])
                    for ft in range(16):
                        gt = 16 * half + ft
                        nc.tensor.matmul(
                            y2_ps[:],
                            w2t[:, 128 * ft : 128 * (ft + 1)],
                            ht[:, TOK * gt : TOK * (gt + 1)],
                            start=(gt == 0),
                            stop=(gt == 31),
                        )
                nc.vector.scalar_tensor_tensor(
                    z2[:, dsl], y2_ps[:], b2_sb[:, dc : dc + 1], xp[:, dsl],
                    ALU.add, ALU.add,
                )
                # interleaved LN2 statistics
                nc.tensor.matmul(
                    sum2_ps[:], ones_sb[:], z2[:, dsl],
                    start=(dc == 0), stop=(dc == 7),
                )
                zsq2 = p_sq.tile([128, TOK], FP32R, name="zsq2", tag="sq")
                nc.scalar.square(zsq2[:], z2[:, dsl])
                nc.tensor.matmul(
                    sq2_ps[:], ones_sb[:], zsq2[:],
                    start=(dc == 0), stop=(dc == 7),
                )

            x_mine = p_xst.tile([128, 8 * TOK], FP32R, name="x_mine", tag="xst")
            if l < L - 1:
                x2b = p_x2b.tile([128, 8 * TOK], BF, name="x2b")
                agi = d_agi.tile([D, TOK], BF, name="agi", tag="agi")
                ago = d_ago.tile(
                    [NC, D, TOK], BF, name="ago", tag="ago", addr_space="Shared"
                )

                def post_dc(dc, _x2b=x2b, _agi=agi, _ago=ago):
                    # ship each normalized d-chunk to the AG bounce as it
                    # completes; fire the single AllGather after the last
                    # (collectives have ~11us fixed cost - do NOT split them)
                    nc.sync.dma_start(
                        _agi[128 * dc : 128 * (dc + 1), :],
                        _x2b[:, TOK * dc : TOK * (dc + 1)],
                    )
                    if dc == 7:
                        nc.gpsimd.collective_compute(
                            "AllGather",
                            ALU.bypass,
                            replica_groups=RG,
                            ins=[_agi[:].opt()],
                            outs=[_ago[:].opt()],
                        )

                _emit_layernorm(
                    ln_env, z2, g2_sb, be2_sb, x_mine, x2b, sum2_ps, sq2_ps,
                    post_dc=post_dc,
                )
                ag_halves = ago
            else:

                def post_dc_out(dc, _x_mine=x_mine):
                    # ship each final d-chunk as soon as it is normalized
                    nc.sync.dma_start(
                        out_e[128 * dc : 128 * (dc + 1), :].bitcast(FP32R),
                        _x_mine[:, TOK * dc : TOK * (dc + 1)],
                    )

                _emit_layernorm(
                    ln_env, z2, g2_sb, be2_sb, x_mine, None, sum2_ps, sq2_ps,
                    post_dc=post_dc_out,
                )

    nc.compile()
    return nc


def _emit_layernorm(env, z, g_sb, b_sb, out_fp, out_bf, sum_ps, sq_ps,
                    post_dc=None):
    """LayerNorm over D; sum/sumsq PSUM stats are pre-accumulated by caller."""
    nc = env["nc"]
    mybir = env["mybir"]
    FP32 = mybir.dt.float32
    FP32R = mybir.dt.float32r
    AF = mybir.ActivationFunctionType
    ALU = mybir.AluOpType
    eps_sb = env["eps"]
    p_stat, p_bc, p_tmp = env["p_stat"], env["p_bc"], env["p_tmp"]

    st = lambda nm: p_stat.tile([1, TOK], FP32, name=nm, tag="lnstat", bufs=8)
    mu = st("mu")
    nc.vector.tensor_scalar_mul(mu[:], sum_ps[:], 1.0 / D)
    ex2 = st("ex2")
    nc.vector.tensor_scalar_mul(ex2[:], sq_ps[:], 1.0 / D)
    var = st("var")
    musq = st("musq")
    nc.vector.tensor_mul(musq[:], mu[:], mu[:])
    nc.vector.tensor_sub(var[:], ex2[:], musq[:])
    std = st("std")
    nc.scalar.activation(std[:], var[:], AF.Sqrt, bias=eps_sb[:], scale=1.0)
    rsig = st("rsig")
    nc.vector.reciprocal_approx_fast(rsig[:], std[:])
    mu_bc = p_bc.tile([128, TOK], FP32, name="mu_bc", tag="mu_bc", bufs=2)
    nc.gpsimd.partition_broadcast(mu_bc[:], mu[:])
    rs_bc = p_bc.tile([128, TOK], FP32, name="rs_bc", tag="rs_bc", bufs=2)
    nc.gpsimd.partition_broadcast(rs_bc[:], rsig[:])
    AF = mybir.ActivationFunctionType
    for dc in range(8):
        dsl = slice(TOK * dc, TOK * (dc + 1))
        t1 = p_tmp.tile([128, TOK], FP32R, name="t1", tag="tmp")
        nc.vector.tensor_sub(t1[:], z[:, dsl], mu_bc[:].bitcast(FP32R))
        t2 = p_tmp.tile([128, TOK], FP32R, name="t2", tag="tmp")
        nc.vector.tensor_mul(t2[:], t1[:], rs_bc[:].bitcast(FP32R))
        # gamma*x + beta on ACT (idle here) - halves the DVE serial tail
        nc.scalar.activation(
            out_fp[:, dsl],
            t2[:],
            AF.Identity,
            bias=b_sb[:, dc : dc + 1],
            scale=g_sb[:, dc : dc + 1],
        )
        if out_bf is not None:
            nc.vector.tensor_copy(out_bf[:, dsl], out_fp[:, dsl])
        if post_dc is not None:
            post_dc(dc)


def _pack_inputs(src, Wq, bq, Wk, bk, Wv, bv, Wo, bo, ln1_g, ln1_b,
                 W1, b1, W2, b2, ln2_g, ln2_b):
    """Host-side sharding/packing. Returns list of per-core input dicts."""
    f32 = np.float32
    # positional encoding (phase == pos, since floor(dim/D) == 0)
    pos = np.arange(S, dtype=f32).reshape(-1, 1)
    dim = np.arange(D)
    pe = np.where(dim[None, :] % 2 == 0, np.sin(pos), np.cos(pos)).astype(f32)
    x0 = (np.asarray(src, f32) + pe[None]).reshape(B * S, D)
    x0T = np.ascontiguousarray(x0.T)                      # [D, 2048]
    x0T_bf = x0T.astype(NPBF16)

    bf = lambda a: np.ascontiguousarray(a).astype(NPBF16)
    fc = lambda a: np.ascontiguousarray(a).astype(f32)

    Wo_p = bf(np.asarray(Wo, f32).reshape(L, 8, 128, 1024))
    bo_p = fc(np.asarray(bo, f32).reshape(L, 8, 128).transpose(0, 2, 1))
    g1_p = fc(np.asarray(ln1_g, f32).reshape(L, 8, 128).transpose(0, 2, 1))
    be1_p = fc(np.asarray(ln1_b, f32).reshape(L, 8, 128).transpose(0, 2, 1))
    g2_p = fc(np.asarray(ln2_g, f32).reshape(L, 8, 128).transpose(0, 2, 1))
    be2_p = fc(np.asarray(ln2_b, f32).reshape(L, 8, 128).transpose(0, 2, 1))
    b1_p = fc(np.asarray(b1, f32).reshape(L, 32, 128).transpose(0, 2, 1))
    b2_p = fc(np.asarray(b2, f32).reshape(L, 8, 128).transpose(0, 2, 1))
    # W1: [L, D, F] -> [L, fc, p, (dt j)]
    W1_p = bf(
        np.asarray(W1, f32)
        .reshape(L, 8, 128, 32, 128)
        .transpose(0, 3, 2, 1, 4)
        .reshape(L, 32, 128, 1024)
    )
    # W2: [L, F, D] -> [L, dc, half, p, (ft j)]
    W2_p = bf(
        np.asarray(W2, f32)
        .reshape(L, 32, 128, 8, 128)
        .transpose(0, 3, 1, 2, 4)      # [L, dc, ft, p, j]
        .reshape(L, 8, 2, 16, 128, 128)
        .transpose(0, 1, 2, 4, 3, 5)   # [L, dc, half, p, ft, j]
        .reshape(L, 8, 2, 128, 2048)
    )
    ident = np.eye(128, dtype=f32)

    Wq = np.asarray(Wq, f32)
    Wk = np.asarray(Wk, f32)
    Wv = np.asarray(Wv, f32)
    bq = np.asarray(bq, f32)
    bk = np.asarray(bk, f32)
    bv = np.asarray(bv, f32)

    def pack_headw(Wx, r):
        # [L, D, 128] for heads 2r, 2r+1 -> [L, 128, (dt j)]
        cat = np.concatenate([Wx[:, 2 * r], Wx[:, 2 * r + 1]], axis=2)  # [L,D,128]
        return bf(
            cat.reshape(L, 8, 128, 128).transpose(0, 2, 1, 3).reshape(L, 128, 1024)
        )

    in_maps = []
    for r in range(NC):
        m = {
            "x0all": x0T_bf,
            "x0mine": np.ascontiguousarray(x0T[:, TOK * r : TOK * (r + 1)]),
            "wq": pack_headw(Wq, r),
            "wk": pack_headw(Wk, r),
            "wv": pack_headw(Wv, r),
            "bq": fc(np.concatenate([bq[:, 2 * r], bq[:, 2 * r + 1]], axis=1))[
                :, :, None
            ],
            "bk": fc(np.concatenate([bk[:, 2 * r], bk[:, 2 * r + 1]], axis=1))[
                :, :, None
            ],
            "bv": fc(np.concatenate([bv[:, 2 * r], bv[:, 2 * r + 1]], axis=1))[
                :, :, None
            ],
            "wo": Wo_p,
            "bo": bo_p,
            "w1": W1_p,
            "b1": b1_p,
            "w2": W2_p,
            "b2": b2_p,
            "g1": g1_p,
            "be1": be1_p,
            "g2": g2_p,
            "be2": be2_p,
            "ident": ident,
        }
        in_maps.append(m)
    return in_maps


def run(inputs, trace=False, trace_kwargs=None):
    """Build (cached), execute on 8 cores, return (output, BassKernelResults)."""
    from concourse.bass_utils import run_bass_kernel_spmd

    if "prog" not in _CACHE:
        _CACHE["prog"] = _build_program()
    nc = _CACHE["prog"]
    in_maps = _pack_inputs(**inputs)
    res = run_bass_kernel_spmd(
        nc, in_maps, list(range(NC)), trace=trace, **(trace_kwargs or {})
    )
    xT = np.empty((B * S, D), np.float32)
    for r in range(NC):
        xT[TOK * r : TOK * (r + 1)] = res.results[r]["out_xT"].T
    return xT.reshape(B, S, D), res


def kernel(**inputs):
    out, _ = run(inputs, trace=False)
    return out



# revision 26
# speedup vs baseline: 1.0921x; 1.0165x over previous
"""Trainium2 Bass kernel for a 6-layer transformer encoder (nn_Encoder).

Sharding across 8 NeuronCores:
  - Attention is tensor-parallel over heads: core r owns heads 2r, 2r+1 and
    computes Q/K/V + scores + softmax + att@V for those heads over ALL tokens.
  - Everything row-wise (out-projection, residuals, LayerNorms, FFN) is
    token-parallel: core r owns tokens [256r, 256r+256) of the flattened
    (batch*seq) axis, with the full Wo/W1/W2 replicated.
  - Per layer only two cheap collectives are needed:
      AllGather of the layer input x^T (bf16, split in two halves so the
      second half overlaps the first half's QKV matmuls)
      AllToAll of per-head attention outputs   -> head-sharded -> token-sharded
    (No AllReduce anywhere: contractions over sharded axes are avoided by
    construction.)

Activations live transposed ("T-layout", [feature, token]) so every matmul
consumes natural-layout weights as the stationary operand. LayerNorm
reductions over D (the partition axis) use ones-matmuls on the PE;
mean/rstd are broadcast back across partitions with gpsimd.partition_broadcast.
Softmax skips max-subtraction (scores are bounded, |S|<6) and its denominator
is computed for free by appending a ones-column to V in the att@V matmul.

dtypes: bf16 for the big GEMMs (QKV / out-proj / FFN weights and moving
operands; fp32 PSUM accumulate), float32r (full-speed fp32, ~1e-4) for
scores / att@V / LN statistics.
"""
import numpy as np
import ml_dtypes

L, H, D, DK, F = 6, 16, 1024, 64, 4096
B, S = 2, 1024
NC = 8
TOK = (B * S) // NC  # 256 tokens per core
HPC = H // NC        # 2 heads per core
EPS = 1e-5
NPBF16 = ml_dtypes.bfloat16

_CACHE = {}


def _build_program():
    import concourse.bacc as bacc
    import concourse.tile as tile
    import concourse.mybir as mybir
    from contextlib import ExitStack

    FP32 = mybir.dt.float32
    FP32R = mybir.dt.float32r
    BF = mybir.dt.bfloat16
    AF = mybir.ActivationFunctionType
    ALU = mybir.AluOpType

    nc = bacc.Bacc(
        "TRN2",
        target_bir_lowering=False,
        debug=False,
        enable_asserts=False,
        num_devices=NC,
    )

    # ---------------- external I/O ----------------
    x0all_e = nc.dram_tensor("x0all", [D, B * S], BF, kind="ExternalInput")
    x0mine_e = nc.dram_tensor("x0mine", [D, TOK], FP32, kind="ExternalInput")
    wq_e = nc.dram_tensor("wq", [L, 128, 1024], BF, kind="ExternalInput")
    wk_e = nc.dram_tensor("wk", [L, 128, 1024], BF, kind="ExternalInput")
    wv_e = nc.dram_tensor("wv", [L, 128, 1024], BF, kind="ExternalInput")
    bq_e = nc.dram_tensor("bq", [L, 128, 1], FP32, kind="ExternalInput")
    bk_e = nc.dram_tensor("bk", [L, 128, 1], FP32, kind="ExternalInput")
    bv_e = nc.dram_tensor("bv", [L, 128, 1], FP32, kind="ExternalInput")
    wo_e = nc.dram_tensor("wo", [L, 8, 128, 1024], BF, kind="ExternalInput")
    bo_e = nc.dram_tensor("bo", [L, 128, 8], FP32, kind="ExternalInput")
    w1_e = nc.dram_tensor("w1", [L, 32, 128, 1024], BF, kind="ExternalInput")
    b1_e = nc.dram_tensor("b1", [L, 128, 32], FP32, kind="ExternalInput")
    w2_e = nc.dram_tensor("w2", [L, 8, 2, 128, 2048], BF, kind="ExternalInput")
    b2_e = nc.dram_tensor("b2", [L, 128, 8], FP32, kind="ExternalInput")
    g1_e = nc.dram_tensor("g1", [L, 128, 8], FP32, kind="ExternalInput")
    be1_e = nc.dram_tensor("be1", [L, 128, 8], FP32, kind="ExternalInput")
    g2_e = nc.dram_tensor("g2", [L, 128, 8], FP32, kind="ExternalInput")
    be2_e = nc.dram_tensor("be2", [L, 128, 8], FP32, kind="ExternalInput")
    ident_e = nc.dram_tensor("ident", [128, 128], FP32, kind="ExternalInput")
    out_e = nc.dram_tensor("out_xT", [D, TOK], FP32, kind="ExternalOutput")

    RG = [list(range(NC))]

    with tile.TileContext(nc) as tc, ExitStack() as ctx:
        P = lambda name, bufs, **kw: ctx.enter_context(
            tc.tile_pool(name=name, bufs=bufs, **kw)
        )
        p_xg = P("xg", 8)        # [128,1024] bf16: x^T d-tiles (one token-half)
        p_qk = P("qk", 2)        # qt/kt [128,2048] fp32r
        p_vtT = P("vtT", 2)      # [128,512] fp32r v^T staging
        p_vaug = P("vaug", 1)    # [128,2080] fp32r: v-natural + ones cols
        p_exp = P("exp", 4)      # [128,1024] fp32r exp(scores^T), 2 heads packed
        p_ota = P("ota", 1)      # [128,2048] bf16: o^T (my heads, all tokens)
        p_otf = P("otf", 1)      # [128,2048] bf16: o^T (all heads, my tokens)
        p_xst = P("xst", 2)      # x_mine [128,2048] fp32r (rotates per layer)
        p_z = P("z", 1)          # residual sums [128,2048] fp32r
        p_xp = P("xp", 1)        # x' [128,2048] fp32r
        p_xpb = P("xpb", 1)      # x' [128,2048] bf16
        p_x2b = P("x2b", 1)      # x'' [128,2048] bf16 (AG payload)
        p_ht = P("ht", 1)        # [128,8192] bf16: FFN hidden (32 x 256)
        p_wqkv = P("wqkv", 3)    # [128,1024] bf16 (3/layer, tag-shared)
        p_wo = P("wo", 8)        # [128,1024] bf16 (8/layer resident)
        p_w1 = P("w1", 3)        # [128,2048] bf16 stream (2 f-chunks per tile)
        p_w2 = P("w2", 4)        # [128,2048] bf16 stream
        p_bias = P("bias", 2)    # small per-layer bias tiles
        p_stat = P("stat", 4)    # [1,N] stats
        p_bc = P("bc", 4)        # broadcast tiles
        p_sq = P("sq", 3)        # z^2 staging [128,256]
        p_tmp = P("tmp", 4)      # [128,256] fp32r temps
        ps_big = P("ps_big", 2, space="PSUM")  # [128,1024] 2-bank slots (shared tag)
        ps_b = P("ps_b", 4, space="PSUM")    # attV [65,512] / transpose / LN stats
        d_agi = P("d_agi", 3, space="DRAM")
        d_ago = P("d_ago", 3, space="DRAM")
        d_a2i = P("d_a2i", 3, space="DRAM")
        d_a2o = P("d_a2o", 3, space="DRAM")

        # constants
        p_const = ctx.enter_context(tc.tile_pool(name="const", bufs=1))
        ones_f = p_const.tile([128, 1], FP32, name="ones_f", tag="onesf")
        nc.gpsimd.memset(ones_f[:], 1.0)
        ones_sb = p_const.tile([128, 1], FP32R, name="ones_sb", tag="ones")
        nc.scalar.activation(ones_sb[:], ones_f[:], AF.Copy, bias=0.0, scale=1.0)
        eps_sb = p_const.tile([1, 1], FP32, name="eps_sb", tag="eps")
        nc.gpsimd.memset(eps_sb[:], float(EPS))
        ident_sb = p_bias.tile([128, 128], FP32R, name="ident_sb", bufs=1)
        nc.sync.dma_start(ident_sb[:], ident_e[:].bitcast(FP32R))

        # warmup collectives: absorb rank-start skew / cold comm-channel cost
        # (~50us each on the first real collectives otherwise) while layer-0
        # QKV+attention compute runs
        wu_i = d_agi.tile([128, 8], BF, name="wu_i", tag="wu_i")
        wu_o = d_ago.tile(
            [NC, 128, 8], BF, name="wu_o", tag="wu_o", addr_space="Shared"
        )
        nc.gpsimd.collective_compute(
            "AllGather", mybir.AluOpType.bypass, replica_groups=RG,
            ins=[wu_i[:].opt()], outs=[wu_o[:].opt()],
        )
        wu2_i = d_a2i.tile([NC, 128, 1], BF, name="wu2_i", tag="wu2_i")
        wu2_o = d_a2o.tile([NC, 128, 1], BF, name="wu2_o", tag="wu2_o")
        nc.gpsimd.collective_compute(
            "AllToAll", mybir.AluOpType.bypass, replica_groups=RG,
            ins=[wu2_i[:].opt()], outs=[wu2_o[:].opt()],
        )

        # initial x_mine (fp32 residual basis)
        x_mine = p_xst.tile([128, 8 * TOK], FP32R, name="x_mine", tag="xst")
        for dc in range(8):
            nc.sync.dma_start(
                x_mine[:, TOK * dc : TOK * (dc + 1)],
                x0mine_e[128 * dc : 128 * (dc + 1), :].bitcast(FP32R),
            )

        ag_halves = None  # two DRAM tensors [NC, 512, TOK] from the split AG

        ln_env = dict(
            nc=nc, mybir=mybir, ones=ones_sb, eps=eps_sb,
            p_sq=p_sq, p_stat=p_stat, p_bc=p_bc, p_tmp=p_tmp, ps_st=ps_b,
        )

        for l in range(L):
            # -- per-layer weight/bias loads (emitted early: prefetch) --
            wq_sb = p_wqkv.tile([128, 1024], BF, name="wq_sb", tag="wqkv")
            nc.sync.dma_start(wq_sb[:], wq_e[l])
            wk_sb = p_wqkv.tile([128, 1024], BF, name="wk_sb", tag="wqkv")
            nc.sync.dma_start(wk_sb[:], wk_e[l])
            wv_sb = p_wqkv.tile([128, 1024], BF, name="wv_sb", tag="wqkv")
            nc.sync.dma_start(wv_sb[:], wv_e[l])
            bq_sb = p_bias.tile([128, 1], FP32, name="bq_sb", tag="bq")
            nc.sync.dma_start(bq_sb[:], bq_e[l])
            bk_sb = p_bias.tile([128, 1], FP32, name="bk_sb", tag="bk")
            nc.sync.dma_start(bk_sb[:], bk_e[l])
            bv_sb = p_bias.tile([128, 1], FP32, name="bv_sb", tag="bv")
            nc.sync.dma_start(bv_sb[:], bv_e[l])
            wo_sb = []
            for t in range(8):
                w = p_wo.tile([128, 1024], BF, name=f"wo{t}", tag="wo")
                nc.sync.dma_start(w[:], wo_e[l, t])
                wo_sb.append(w)
            bo_sb = p_bias.tile([128, 8], FP32, name="bo_sb", tag="bo")
            nc.sync.dma_start(bo_sb[:], bo_e[l])
            g1_sb = p_bias.tile([128, 8], FP32, name="g1_sb", tag="g1")
            nc.sync.dma_start(g1_sb[:], g1_e[l])
            be1_sb = p_bias.tile([128, 8], FP32, name="be1_sb", tag="be1")
            nc.sync.dma_start(be1_sb[:], be1_e[l])
            b1_sb = p_bias.tile([128, 32], FP32, name="b1_sb", tag="b1")
            nc.sync.dma_start(b1_sb[:], b1_e[l])
            b2_sb = p_bias.tile([128, 8], FP32, name="b2_sb", tag="b2")
            nc.sync.dma_start(b2_sb[:], b2_e[l])
            g2_sb = p_bias.tile([128, 8], FP32, name="g2_sb", tag="g2")
            nc.sync.dma_start(g2_sb[:], g2_e[l])
            be2_sb = p_bias.tile([128, 8], FP32, name="be2_sb", tag="be2")
            nc.sync.dma_start(be2_sb[:], be2_e[l])

            # ---------- QKV ----------
            # bf16 for q/k/exp/vaug: HW runs fp32r matmuls in fp32_mode=HIGH
            # (~4 cyc/row); bf16 is 1 cyc/row and the 2e-2 tolerance allows it.
            qt = p_qk.tile([128, 2048], BF, name="qt", tag="qk")
            kt = p_qk.tile([128, 2048], BF, name="kt", tag="qk")
            vaug = p_vaug.tile([128, 16 * (2 * DK + 2)], BF, name="vaug")
            VW = 2 * DK + 2  # 130: per t-tile block [h0(64)|ones|h1(64)|ones]
            # pre-fill the ones columns once (gpsimd, off the DVE queue)
            nc.gpsimd.memset(vaug[:], 1.0)

            for half in range(2):
                # xg: one token-half (= one batch) of full x^T, 8 d-tiles
                xg = []
                for dt in range(8):
                    t = p_xg.tile([128, 1024], BF, name=f"xg{dt}", tag="xg")
                    if l == 0:
                        nc.sync.dma_start(
                            t[:],
                            x0all_e[
                                128 * dt : 128 * (dt + 1),
                                1024 * half : 1024 * (half + 1),
                            ],
                        )
                    else:
                        # token-half `half` = rank blocks 4*half..4*half+3
                        nc.sync.dma_start(
                            t[:].rearrange("p (r s) -> p r s", r=4),
                            ag_halves[
                                4 * half : 4 * half + 4,
                                128 * dt : 128 * (dt + 1),
                                :,
                            ].rearrange("r p s -> p r s"),
                        )
                    xg.append(t)
                for sc2 in range(2):
                    sc = 2 * half + sc2
                    ssl = slice(512 * sc, 512 * (sc + 1))  # global token chunk
                    hsl = slice(512 * sc2, 512 * (sc2 + 1))  # within-half chunk
                    q_ps = ps_big.tile([128, 512], FP32, name="q_ps", tag="big")
                    for dt in range(8):
                        nc.tensor.matmul(
                            q_ps[:],
                            wq_sb[:, 128 * dt : 128 * (dt + 1)],
                            xg[dt][:, hsl],
                            start=(dt == 0),
                            stop=(dt == 7),
                        )
                    nc.vector.tensor_scalar_add(qt[:, ssl], q_ps[:], bq_sb[:])
                    k_ps = ps_big.tile([128, 512], FP32, name="k_ps", tag="big")
                    for dt in range(8):
                        nc.tensor.matmul(
                            k_ps[:],
                            wk_sb[:, 128 * dt : 128 * (dt + 1)],
                            xg[dt][:, hsl],
                            start=(dt == 0),
                            stop=(dt == 7),
                        )
                    nc.vector.tensor_scalar_add(kt[:, ssl], k_ps[:], bk_sb[:])
                    v_ps = ps_big.tile([128, 512], FP32, name="v_ps", tag="big")
                    for dt in range(8):
                        nc.tensor.matmul(
                            v_ps[:],
                            wv_sb[:, 128 * dt : 128 * (dt + 1)],
                            xg[dt][:, hsl],
                            start=(dt == 0),
                            stop=(dt == 7),
                        )
                    vtT = p_vtT.tile([128, 512], FP32R, name="vtT", tag="vtT")
                    nc.vector.tensor_scalar_add(vtT[:], v_ps[:], bv_sb[:])
                    for j in range(4):
                        tt = 4 * sc + j  # global t-tile 0..15
                        tr_ps = ps_b.tile([128, 128], FP32, name="tr_ps", tag="ps")
                        nc.tensor.transpose(
                            tr_ps[:].bitcast(FP32R),
                            vtT[:, 128 * j : 128 * (j + 1)],
                            ident_sb[:],
                        )
                        o = VW * tt
                        nc.vector.tensor_copy(vaug[:, o : o + 64], tr_ps[:, 0:64])
                        nc.vector.tensor_copy(
                            vaug[:, o + 65 : o + 129], tr_ps[:, 64:128]
                        )

            # ---------- attention ----------
            # Two independent 512-token streams (the sc chunks of one batch)
            # interleave so the PE never waits on the scores->exp->attV chain.
            ota = p_ota.tile([128, 2048], BF, name="ota")
            a2i = d_a2i.tile([NC, 128, TOK], BF, name="a2i", tag="a2i")
            for b in range(B):
                base = S * b
                ssls = [
                    slice(base + 512 * sc, base + 512 * (sc + 1)) for sc in range(2)
                ]
                o_ps = {
                    (sc, h): ps_b.tile([65, 512], FP32, name=f"o_ps{sc}{h}", tag="ps")
                    for sc in range(2)
                    for h in range(HPC)
                }
                e_prev = {0: None, 1: None}
                for jt in range(9):
                    for sc in range(2):
                        if jt < 8:
                            tsl = slice(base + 128 * jt, base + 128 * (jt + 1))
                            # both heads' scores into one 2-bank PSUM tile,
                            # then a single [128,1024] exp on ACT
                            s2 = ps_big.tile(
                                [128, 1024], FP32, name="s2", tag="big"
                            )
                            for h in range(HPC):
                                hp = slice(64 * h, 64 * (h + 1))
                                nc.tensor.matmul(
                                    s2[:, 512 * h : 512 * (h + 1)],
                                    kt[hp, tsl],
                                    qt[hp, ssls[sc]],
                                    start=True,
                                    stop=True,
                                )
                            e2 = p_exp.tile(
                                [128, 1024], BF, name="e2", tag="e"
                            )
                            nc.scalar.activation(
                                e2[:], s2[:], AF.Exp, bias=0.0, scale=0.125
                            )
                        if jt > 0:
                            pj = jt - 1
                            for h in range(HPC):
                                o = VW * (8 * b + pj) + (DK + 1) * h
                                nc.tensor.matmul(
                                    o_ps[(sc, h)][:],
                                    vaug[:, o : o + 65],
                                    e_prev[sc][:, 512 * h : 512 * (h + 1)],
                                    start=(pj == 0),
                                    stop=(pj == 7),
                                )
                        e_prev[sc] = e2 if jt < 8 else None
                for sc in range(2):
                    for h in range(HPC):
                        drow = p_stat.tile(
                            [1, 512], FP32, name="drow", tag="drow", bufs=2
                        )
                        nc.vector.tensor_copy(drow[:], o_ps[(sc, h)][64:65, :])
                        den = p_stat.tile(
                            [1, 512], FP32, name="den", tag="den", bufs=2
                        )
                        nc.vector.reciprocal_approx_fast(den[:], drow[:])
                        den_bc = p_bc.tile(
                            [64, 512], FP32, name="den_bc", tag="dbc", bufs=2
                        )
                        nc.gpsimd.partition_broadcast(den_bc[:], den[:])
                        nc.vector.tensor_mul(
                            ota[64 * h : 64 * (h + 1), ssls[sc]],
                            o_ps[(sc, h)][0:64, :],
                            den_bc[:].bitcast(FP32R),
                        )
                    # ship this 512-token quadrant (= 2 rank chunks) to the
                    # A2A bounce as soon as both heads are normalized
                    rr = 2 * (2 * b + sc)
                    nc.sync.dma_start(
                        a2i[rr : rr + 2].rearrange("r p s -> p r s"),
                        ota[:, ssls[sc]].rearrange("p (r s) -> p r s", r=2),
                    )

            # ---------- AllToAll + out-projection + LN1 ----------
            a2o = d_a2o.tile([NC, 128, TOK], BF, name="a2o", tag="a2o")
            nc.gpsimd.collective_compute(
                "AllToAll",
                ALU.bypass,
                replica_groups=RG,
                ins=[a2i[:].opt()],
                outs=[a2o[:].opt()],
            )
            otf = p_otf.tile([128, 8 * TOK], BF, name="otf")
            for t in range(NC):
                nc.sync.dma_start(otf[:, TOK * t : TOK * (t + 1)], a2o[t])

            z1 = p_z.tile([128, 8 * TOK], FP32R, name="z1", tag="z")
            sum1_ps = ps_b.tile([1, TOK], FP32, name="sum1_ps", tag="ps")
            sq1_ps = ps_b.tile([1, TOK], FP32, name="sq1_ps", tag="ps")
            for dc in range(8):
                dsl = slice(TOK * dc, TOK * (dc + 1))
                y_ps = ps_big.tile([128, TOK], FP32, name="y_ps", tag="big")
                for t in range(8):
                    nc.tensor.matmul(
                        y_ps[:],
                        wo_sb[t][:, 128 * dc : 128 * (dc + 1)],
                        otf[:, TOK * t : TOK * (t + 1)],
                        start=(t == 0),
                        stop=(t == 7),
                    )
                nc.vector.scalar_tensor_tensor(
                    z1[:, dsl], y_ps[:], bo_sb[:, dc : dc + 1], x_mine[:, dsl],
                    ALU.add, ALU.add,
                )
                # interleaved LN1 statistics
                nc.tensor.matmul(
                    sum1_ps[:], ones_sb[:], z1[:, dsl],
                    start=(dc == 0), stop=(dc == 7),
                )
                zsq = p_sq.tile([128, TOK], FP32R, name="zsq", tag="sq")
                nc.vector.tensor_mul(zsq[:], z1[:, dsl], z1[:, dsl])
                nc.tensor.matmul(
                    sq1_ps[:], ones_sb[:], zsq[:],
                    start=(dc == 0), stop=(dc == 7),
                )

            xp = p_xp.tile([128, 8 * TOK], FP32R, name="xp")
            xpb = p_xpb.tile([128, 8 * TOK], BF, name="xpb")
            _emit_layernorm(ln_env, z1, g1_sb, be1_sb, xp, xpb, sum1_ps, sq1_ps)

            # ---------- FFN + LN2 (+ split AllGather for next layer) ----------
            ht = p_ht.tile([128, 32 * TOK], BF, name="ht")
            for g in range(16):
                w1t = p_w1.tile([128, 2048], BF, name="w1t", tag="w1")
                nc.sync.dma_start(
                    w1t[:].rearrange("p (c j) -> p c j", c=2),
                    w1_e[l, 2 * g : 2 * g + 2].rearrange("c p j -> p c j"),
                )
                for c in range(2):
                    fc = 2 * g + c
                    h_ps = ps_big.tile([128, TOK], FP32, name="h_ps", tag="big")
                    for dt in range(8):
                        nc.tensor.matmul(
                            h_ps[:],
                            w1t[:, 1024 * c + 128 * dt : 1024 * c + 128 * (dt + 1)],
                            xpb[:, TOK * dt : TOK * (dt + 1)],
                            start=(dt == 0),
                            stop=(dt == 7),
                        )
                    # relu(h + b1) on DVE (keeps ACT free for attention exp)
                    nc.vector.tensor_scalar(
                        ht[:, TOK * fc : TOK * (fc + 1)],
                        h_ps[:],
                        b1_sb[:, fc : fc + 1],
                        0.0,
                        ALU.add,
                        ALU.max,
                    )

            z2 = p_z.tile([128, 8 * TOK], FP32R, name="z2", tag="z")
            sum2_ps = ps_b.tile([1, TOK], FP32, name="sum2_ps", tag="ps")
            sq2_ps = ps_b.tile([1, TOK], FP32, name="sq2_ps", tag="ps")
            for dc in range(8):
                dsl = slice(TOK * dc, TOK * (dc + 1))
                y2_ps = ps_big.tile([128, TOK], FP32, name="y2_ps", tag="big")
                for half in range(2):
                    w2t = p_w2.tile([128, 2048], BF, name="w2t", tag="w2")
                    nc.sync.dma_start(w2t[:], w2_e[l, dc, half])
                    for ft in range(16):
                        gt = 16 * half + ft
                        nc.tensor.matmul(
                            y2_ps[:],
                            w2t[:, 128 * ft : 128 * (ft + 1)],
                            ht[:, TOK * gt : TOK * (gt + 1)],
                            start=(gt == 0),
                            stop=(gt == 31),
                        )
                nc.vector.scalar_tensor_tensor(
                    z2[:, dsl], y2_ps[:], b2_sb[:, dc : dc + 1], xp[:, dsl],
                    ALU.add, ALU.add,
                )
                # interleaved LN2 statistics
                nc.tensor.matmul(
                    sum2_ps[:], ones_sb[:], z2[:, dsl],
                    start=(dc == 0), stop=(dc == 7),
                )
                zsq2 = p_sq.tile([128, TOK], FP32R, name="zsq2", tag="sq")
                nc.vector.tensor_mul(zsq2[:], z2[:, dsl], z2[:, dsl])
                nc.tensor.matmul(
                    sq2_ps[:], ones_sb[:], zsq2[:],
                    start=(dc == 0), stop=(dc == 7),
                )

            x_mine = p_xst.tile([128, 8 * TOK], FP32R, name="x_mine", tag="xst")
            if l < L - 1:
                x2b = p_x2b.tile([128, 8 * TOK], BF, name="x2b")
                agi = d_agi.tile([D, TOK], BF, name="agi", tag="agi")
                ago = d_ago.tile(
                    [NC, D, TOK], BF, name="ago", tag="ago", addr_space="Shared"
                )

                def post_dc(dc, _x2b=x2b, _agi=agi, _ago=ago):
                    # ship each normalized d-chunk to the AG bounce as it
                    # completes; fire the single AllGather after the last
                    # (collectives have ~11us fixed cost - do NOT split them)
                    nc.sync.dma_start(
                        _agi[128 * dc : 128 * (dc + 1), :],
                        _x2b[:, TOK * dc : TOK * (dc + 1)],
                    )
                    if dc == 7:
                        nc.gpsimd.collective_compute(
                            "AllGather",
                            ALU.bypass,
                            replica_groups=RG,
                            ins=[_agi[:].opt()],
                            outs=[_ago[:].opt()],
                        )

                _emit_layernorm(
                    ln_env, z2, g2_sb, be2_sb, x_mine, x2b, sum2_ps, sq2_ps,
                    post_dc=post_dc,
                )
                ag_halves = ago
            else:

                def post_dc_out(dc, _x_mine=x_mine):
                    # ship each final d-chunk as soon as it is normalized
                    nc.sync.dma_start(
                        out_e[128 * dc : 128 * (dc + 1), :].bitcast(FP32R),
                        _x_mine[:, TOK * dc : TOK * (dc + 1)],
                    )

                _emit_layernorm(
                    ln_env, z2, g2_sb, be2_sb, x_mine, None, sum2_ps, sq2_ps,
                    post_dc=post_dc_out,
                )

    nc.compile()
    return nc


def _emit_layernorm(env, z, g_sb, b_sb, out_fp, out_bf, sum_ps, sq_ps,
                    post_dc=None):
    """LayerNorm over D; sum/sumsq PSUM stats are pre-accumulated by caller."""
    nc = env["nc"]
    mybir = env["mybir"]
    FP32 = mybir.dt.float32
    FP32R = mybir.dt.float32r
    AF = mybir.ActivationFunctionType
    ALU = mybir.AluOpType
    eps_sb = env["eps"]
    p_stat, p_bc, p_tmp = env["p_stat"], env["p_bc"], env["p_tmp"]

    st = lambda nm: p_stat.tile([1, TOK], FP32, name=nm, tag="lnstat", bufs=8)
    mu = st("mu")
    nc.vector.tensor_scalar_mul(mu[:], sum_ps[:], 1.0 / D)
    ex2 = st("ex2")
    nc.vector.tensor_scalar_mul(ex2[:], sq_ps[:], 1.0 / D)
    var = st("var")
    musq = st("musq")
    nc.vector.tensor_mul(musq[:], mu[:], mu[:])
    nc.vector.tensor_sub(var[:], ex2[:], musq[:])
    std = st("std")
    nc.scalar.activation(std[:], var[:], AF.Sqrt, bias=eps_sb[:], scale=1.0)
    rsig = st("rsig")
    nc.vector.reciprocal_approx_fast(rsig[:], std[:])
    mu_bc = p_bc.tile([128, TOK], FP32, name="mu_bc", tag="mu_bc", bufs=2)
    nc.gpsimd.partition_broadcast(mu_bc[:], mu[:])
    rs_bc = p_bc.tile([128, TOK], FP32, name="rs_bc", tag="rs_bc", bufs=2)
    nc.gpsimd.partition_broadcast(rs_bc[:], rsig[:])
    for dc in range(8):
        dsl = slice(TOK * dc, TOK * (dc + 1))
        t1 = p_tmp.tile([128, TOK], FP32R, name="t1", tag="tmp")
        nc.vector.tensor_sub(t1[:], z[:, dsl], mu_bc[:].bitcast(FP32R))
        t2 = p_tmp.tile([128, TOK], FP32R, name="t2", tag="tmp")
        nc.vector.tensor_mul(t2[:], t1[:], rs_bc[:].bitcast(FP32R))
        nc.vector.tensor_scalar(
            out_fp[:, dsl],
            t2[:],
            g_sb[:, dc : dc + 1],
            b_sb[:, dc : dc + 1],
            ALU.mult,
            ALU.add,
        )
        if out_bf is not None:
            nc.vector.tensor_copy(out_bf[:, dsl], out_fp[:, dsl])
        if post_dc is not None:
            post_dc(dc)


def _pack_inputs(src, Wq, bq, Wk, bk, Wv, bv, Wo, bo, ln1_g, ln1_b,
                 W1, b1, W2, b2, ln2_g, ln2_b):
    """Host-side sharding/packing. Returns list of per-core input dicts."""
    f32 = np.float32
    # positional encoding (phase == pos, since floor(dim/D) == 0)
    pos = np.arange(S, dtype=f32).reshape(-1, 1)
    dim = np.arange(D)
    pe = np.where(dim[None, :] % 2 == 0, np.sin(pos), np.cos(pos)).astype(f32)
    x0 = (np.asarray(src, f32) + pe[None]).reshape(B * S, D)
    x0T = np.ascontiguousarray(x0.T)                      # [D, 2048]
    x0T_bf = x0T.astype(NPBF16)

    bf = lambda a: np.ascontiguousarray(a).astype(NPBF16)
    fc = lambda a: np.ascontiguousarray(a).astype(f32)

    Wo_p = bf(np.asarray(Wo, f32).reshape(L, 8, 128, 1024))
    bo_p = fc(np.asarray(bo, f32).reshape(L, 8, 128).transpose(0, 2, 1))
    g1_p = fc(np.asarray(ln1_g, f32).reshape(L, 8, 128).transpose(0, 2, 1))
    be1_p = fc(np.asarray(ln1_b, f32).reshape(L, 8, 128).transpose(0, 2, 1))
    g2_p = fc(np.asarray(ln2_g, f32).reshape(L, 8, 128).transpose(0, 2, 1))
    be2_p = fc(np.asarray(ln2_b, f32).reshape(L, 8, 128).transpose(0, 2, 1))
    b1_p = fc(np.asarray(b1, f32).reshape(L, 32, 128).transpose(0, 2, 1))
    b2_p = fc(np.asarray(b2, f32).reshape(L, 8, 128).transpose(0, 2, 1))
    # W1: [L, D, F] -> [L, fc, p, (dt j)]
    W1_p = bf(
        np.asarray(W1, f32)
        .reshape(L, 8, 128, 32, 128)
        .transpose(0, 3, 2, 1, 4)
        .reshape(L, 32, 128, 1024)
    )
    # W2: [L, F, D] -> [L, dc, half, p, (ft j)]
    W2_p = bf(
        np.asarray(W2, f32)
        .reshape(L, 32, 128, 8, 128)
        .transpose(0, 3, 1, 2, 4)      # [L, dc, ft, p, j]
        .reshape(L, 8, 2, 16, 128, 128)
        .transpose(0, 1, 2, 4, 3, 5)   # [L, dc, half, p, ft, j]
        .reshape(L, 8, 2, 128, 2048)
    )
    ident = np.eye(128, dtype=f32)

    Wq = np.asarray(Wq, f32)
    Wk = np.asarray(Wk, f32)
    Wv = np.asarray(Wv, f32)
    bq = np.asarray(bq, f32)
    bk = np.asarray(bk, f32)
    bv = np.asarray(bv, f32)

    def pack_headw(Wx, r):
        # [L, D, 128] for heads 2r, 2r+1 -> [L, 128, (dt j)]
        cat = np.concatenate([Wx[:, 2 * r], Wx[:, 2 * r + 1]], axis=2)  # [L,D,128]
        return bf(
            cat.reshape(L, 8, 128, 128).transpose(0, 2, 1, 3).reshape(L, 128, 1024)
        )

    in_maps = []
    for r in range(NC):
        m = {
            "x0all": x0T_bf,
            "x0mine": np.ascontiguousarray(x0T[:, TOK * r : TOK * (r + 1)]),
            "wq": pack_headw(Wq, r),
            "wk": pack_headw(Wk, r),
            "wv": pack_headw(Wv, r),
            "bq": fc(np.concatenate([bq[:, 2 * r], bq[:, 2 * r + 1]], axis=1))[
                :, :, None
            ],
            "bk": fc(np.concatenate([bk[:, 2 * r], bk[:, 2 * r + 1]], axis=1))[
                :, :, None
            ],
            "bv": fc(np.concatenate([bv[:, 2 * r], bv[:, 2 * r + 1]], axis=1))[
                :, :, None
            ],
            "wo": Wo_p,
            "bo": bo_p,
            "w1": W1_p,
            "b1": b1_p,
            "w2": W2_p,
            "b2": b2_p,
            "g1": g1_p,
            "be1": be1_p,
            "g2": g2_p,
            "be2": be2_p,
            "ident": ident,
        }
        in_maps.append(m)
    return in_maps


def run(inputs, trace=False, trace_kwargs=None):
    """Build (cached), execute on 8 cores, return (output, BassKernelResults)."""
    from concourse.bass_utils import run_bass_kernel_spmd

    if "prog" not in _CACHE:
        _CACHE["prog"] = _build_program()
    nc = _CACHE["prog"]
    in_maps = _pack_inputs(**inputs)
    res = run_bass_kernel_spmd(
        nc, in_maps, list(range(NC)), trace=trace, **(trace_kwargs or {})
    )
    xT = np.empty((B * S, D), np.float32)
    for r in range(NC):
        xT[TOK * r : TOK * (r + 1)] = res.results[r]["out_xT"].T
    return xT.reshape(B, S, D), res


def kernel(**inputs):
    out, _ = run(inputs, trace=False)
    return out

